# revision 13
# baseline (speedup 1.0000x reference)
"""Trainium2 Bass kernel for a dense transformer block (pre-LN, 12 heads, MLP 4x).

Strategy: data-parallel over batch across the 8 NeuronCores (B=8 -> one batch
element per core, no collectives). Per core:

  - residual stream kept token-major [128 tok x 768] (8 token chunks)
  - LN on DVE via bn_stats/bn_aggr, (x-mu)*rstd fused in one tensor_scalar
  - LN gains/biases folded into the following matmul weights on the host
  - h transposed to feature-major via PE transpose (48 blocks) to feed matmuls
  - QKV: q,k produced feature-major [64d x tok per head]; v produced
    token-major directly (so P@V needs no on-chip transposes)
  - attention computed transposed: S_t[k,q] = k_fm.T @ q_fm, exp on ACT with
    fused 1/sqrt(d) scale, denominator via an extra ones-row appended to V
    (ctx psum row 64), normalization folded into ctx evacuation
  - MLP: fc1 evacuated by ACT with fused bias+GELU (exact), fc2 accumulates
    token-major, residual added during evacuation
  - all matmuls run as float32r (full PE rate for free-dim >= 256) with fp32
    PSUM accumulation; fp32 storage everywhere
"""

import numpy as np

import concourse.bass as bass
import concourse.mybir as mybir
import concourse.tile as tile
from concourse import bacc
from concourse.masks import make_identity

DIM = 768
HEADS = 12
HD = 64  # head dim
HIDDEN = 3072
N_TOK = 1024
TC = N_TOK // 128  # 8 token chunks
FC = DIM // 128  # 6 feature chunks
MC_QK = 2 * DIM // 128  # 12 chunks of q|k features
MC_H = HIDDEN // 128  # 24 hidden chunks
EPS = 1e-5
SCALE = HD ** -0.5

F32 = mybir.dt.float32
F32R = mybir.dt.float32r


def _r(ap):
    """Bitcast an fp32 AP to float32r for full-rate PE matmul."""
    return ap.bitcast(F32R)


def _ln_chunk(nc, stat_pool, eps_tile, x_ap, out_ap):
    """out = (x - mean(x)) * rsqrt(var(x) + eps), row-wise over 768."""
    stats = stat_pool.tile([128, 3, 6], F32, tag="ln_stats")
    for sg in range(3):
        nc.vector.bn_stats(out=stats[:, sg, :], in_=x_ap[:, sg * 256:(sg + 1) * 256])
    mv = stat_pool.tile([128, 2], F32, tag="ln_mv")
    nc.vector.bn_aggr(out=mv, in_=stats)
    rstd = stat_pool.tile([128, 1], F32, tag="ln_rstd")
    nc.scalar.activation(
        out=rstd, in_=mv[:, 1:2], func=mybir.ActivationFunctionType.Sqrt,
        bias=eps_tile, scale=1.0,
    )
    nc.vector.reciprocal(out=rstd, in_=rstd)
    nc.vector.tensor_scalar(
        out=out_ap, in0=x_ap, scalar1=mv[:, 0:1], scalar2=rstd,
        op0=mybir.AluOpType.subtract, op1=mybir.AluOpType.mult,
    )


def build_bass():
    nc = bacc.Bacc("TRN2", debug=False)

    x_d = nc.dram_tensor("x", [N_TOK, DIM], F32, kind="ExternalInput")
    qkv_wt_d = nc.dram_tensor("qkv_wt", [DIM, 3 * DIM], F32R, kind="ExternalInput")
    qkb_pm_d = nc.dram_tensor("qkb_pm", [128, MC_QK], F32, kind="ExternalInput")
    vb_d = nc.dram_tensor("vb", [DIM], F32, kind="ExternalInput")
    proj_wt_d = nc.dram_tensor("proj_wt", [DIM, DIM], F32R, kind="ExternalInput")
    projb_d = nc.dram_tensor("projb", [DIM], F32, kind="ExternalInput")
    fc1_wt_d = nc.dram_tensor("fc1_wt", [DIM, HIDDEN], F32R, kind="ExternalInput")
    fc1b_pm_d = nc.dram_tensor("fc1b_pm", [128, MC_H], F32, kind="ExternalInput")
    fc2_wt_d = nc.dram_tensor("fc2_wt", [HIDDEN, DIM], F32R, kind="ExternalInput")
    fc2b_d = nc.dram_tensor("fc2b", [DIM], F32, kind="ExternalInput")
    out_d = nc.dram_tensor("out", [N_TOK, DIM], F32, kind="ExternalOutput")

    x_dt = x_d.ap().rearrange("(t p) c -> p t c", p=128)
    out_dt = out_d.ap().rearrange("(t p) c -> p t c", p=128)

    def bcast128(ap_1d, n):
        return bass.AP(tensor=ap_1d.tensor, offset=ap_1d.offset,
                       ap=[[0, 128], [1, n]])

    with tile.TileContext(nc) as tc:
        with (
            tc.tile_pool(name="const", bufs=1) as const_pool,
            tc.tile_pool(name="resid", bufs=1) as resid_pool,
            tc.tile_pool(name="stats", bufs=4) as stat_pool,
            tc.tile_pool(name="psum_mm", bufs=4, space="PSUM") as psum_mm,
        ):
            ident = const_pool.tile([128, 128], F32)
            make_identity(nc, ident)
            eps_tile = const_pool.tile([128, 1], F32)
            nc.vector.memset(eps_tile, EPS)
            ones_f32 = const_pool.tile([128, 128], F32)
            nc.vector.memset(ones_f32, 1.0)
            ones_row = const_pool.tile([1, 128], F32)
            nc.scalar.copy(out=_r(ones_row), in_=ones_f32[0:1, :])
            qkb_pm = const_pool.tile([128, MC_QK], F32)
            nc.sync.dma_start(out=qkb_pm, in_=qkb_pm_d.ap())
            fc1b_pm = const_pool.tile([128, MC_H], F32)
            nc.sync.dma_start(out=fc1b_pm, in_=fc1b_pm_d.ap())
            vb_bc = const_pool.tile([128, DIM], F32)
            nc.sync.dma_start(out=vb_bc, in_=bcast128(vb_d.ap(), DIM))
            projb_bc = const_pool.tile([128, DIM], F32)
            nc.sync.dma_start(out=projb_bc, in_=bcast128(projb_d.ap(), DIM))
            fc2b_bc = const_pool.tile([128, DIM], F32)
            nc.sync.dma_start(out=fc2b_bc, in_=bcast128(fc2b_d.ap(), DIM))

            x_sb = resid_pool.tile([128, TC, DIM], F32)
            nc.sync.dma_start(out=x_sb, in_=x_dt)

            # ---------------- LN1 + transpose -> h_fm [768, 1024] -------------
            with (
                tc.tile_pool(name="qk", bufs=1) as qk_pool,
                tc.tile_pool(name="vaug", bufs=1) as v_pool,
            ):
                qk_fm = qk_pool.tile([128, MC_QK, N_TOK], F32)
                v_aug = v_pool.tile([128, TC, HEADS, HD + 1], F32)

                with (
                    tc.tile_pool(name="hfm", bufs=1) as hfm_pool,
                    tc.tile_pool(name="htmp", bufs=3) as htmp_pool,
                    tc.tile_pool(name="wqkv", bufs=8) as wqkv_pool,
                    tc.tile_pool(name="wv", bufs=8) as wv_pool,
                    tc.tile_pool(name="psum_tr", bufs=2, space="PSUM") as psum_tr,
                ):
                    h_fm = hfm_pool.tile([128, FC, N_TOK], F32, tag="hfm")
                    for t in range(TC):
                        h_tile = htmp_pool.tile([128, DIM], F32, tag="h_tile")
                        _ln_chunk(nc, stat_pool, eps_tile, x_sb[:, t, :], h_tile)
                        for f in range(FC):
                            tr = psum_tr.tile([128, 128], F32, tag="tr")
                            nc.tensor.transpose(
                                tr, h_tile[:, f * 128:(f + 1) * 128], ident)
                            nc.scalar.copy(
                                out=_r(h_fm[:, f, t * 128:(t + 1) * 128]), in_=tr)

                    # ---------------- QKV ----------------
                    # q,k feature-major: qk_fm [128, 12, 1024]
                    for m in range(MC_QK):
                        w_tiles = []
                        for k in range(FC):
                            wt = wqkv_pool.tile([128, 128], F32R, tag="wqkv")
                            nc.sync.dma_start(
                                out=wt,
                                in_=qkv_wt_d.ap()[k * 128:(k + 1) * 128,
                                                  m * 128:(m + 1) * 128])
                            w_tiles.append(wt)
                        for q in range(2):
                            ps = psum_mm.tile([128, 512], F32, tag="mm")
                            for k in range(FC):
                                nc.tensor.matmul(
                                    ps, _r(w_tiles[k]),
                                    _r(h_fm[:, k, q * 512:(q + 1) * 512]),
                                    start=(k == 0), stop=(k == FC - 1))
                            nc.vector.tensor_scalar_add(
                                out=_r(qk_fm[:, m, q * 512:(q + 1) * 512]), in0=ps,
                                scalar1=qkb_pm[:, m:m + 1])

                    # v token-major with ones column: v_aug [128, 8, 12, 65]
                    nc.scalar.copy(
                        out=_r(v_aug[:, :, :, HD]),
                        in_=ones_f32[:, 0:96].rearrange("p (a b) -> p a b", a=TC))
                    for nv in range(2):
                        w_tiles = []
                        for k in range(FC):
                            wt = wv_pool.tile([128, 384], F32R, tag="wv")
                            nc.sync.dma_start(
                                out=wt,
                                in_=qkv_wt_d.ap()[k * 128:(k + 1) * 128,
                                                  2 * DIM + nv * 384:
                                                  2 * DIM + (nv + 1) * 384])
                            w_tiles.append(wt)
                        for t in range(TC):
                            ps = psum_mm.tile([128, 384], F32, tag="mm")
                            for k in range(FC):
                                nc.tensor.matmul(
                                    ps, _r(h_fm[:, k, t * 128:(t + 1) * 128]),
                                    _r(w_tiles[k]),
                                    start=(k == 0), stop=(k == FC - 1))
                            nc.vector.tensor_add(
                                out=_r(v_aug[:, t, nv * 6:(nv + 1) * 6, 0:HD]),
                                in0=ps.rearrange("p (h d) -> p h d", d=HD),
                                in1=vb_bc[:, nv * 384:(nv + 1) * 384].rearrange(
                                    "p (h d) -> p h d", d=HD))

                # ---------------- attention ----------------
                with tc.tile_pool(name="ctxfm", bufs=1) as ctx_pool:
                    ctx_fm = ctx_pool.tile([128, FC, N_TOK], F32)
                    with (
                        tc.tile_pool(name="exps", bufs=10) as exps_pool,
                        tc.tile_pool(name="dsmall", bufs=4) as dsmall_pool,
                        tc.tile_pool(name="psum_ctx", bufs=2, space="PSUM") as psum_ctx,
                    ):
                        for h in range(HEADS):
                            po = 64 * (h % 2)
                            q_fm_h = qk_fm[po:po + 64, h // 2, :]
                            k_fm_h = qk_fm[po:po + 64, 6 + h // 2, :]
                            exps = []
                            for kc in range(TC):
                                e_t = exps_pool.tile([128, N_TOK], F32, tag="exps")
                                for q in range(2):
                                    sp = psum_mm.tile([128, 512], F32, tag="mm")
                                    nc.tensor.matmul(
                                        sp, _r(k_fm_h[:, kc * 128:(kc + 1) * 128]),
                                        _r(q_fm_h[:, q * 512:(q + 1) * 512]),
                                        start=True, stop=True)
                                    nc.scalar.activation(
                                        out=_r(e_t[:, q * 512:(q + 1) * 512]), in_=sp,
                                        func=mybir.ActivationFunctionType.Exp,
                                        scale=SCALE)
                                exps.append(e_t)
                            for q in range(2):
                                cp = psum_ctx.tile([65, 512], F32, tag="ctx")
                                for kc in range(TC):
                                    nc.tensor.matmul(
                                        cp, _r(v_aug[:, kc, h, :]),
                                        _r(exps[kc][:, q * 512:(q + 1) * 512]),
                                        start=(kc == 0), stop=(kc == TC - 1))
                                recip = dsmall_pool.tile([1, 512], F32, tag="recip")
                                with nc.allow_low_precision(
                                        reason="f32r rounding of softmax recip"):
                                    nc.vector.reciprocal(
                                        out=_r(recip), in_=cp[64:65, :])
                                # broadcast 1/denom across the 64 head-dim
                                # partitions via a K=1 ones matmul
                                bc = psum_ctx.tile([64, 512], F32, tag="bc")
                                nc.tensor.matmul(
                                    bc, _r(ones_row[0:1, 0:64]), _r(recip),
                                    start=True, stop=True)
                                ctx_sl = ctx_fm[po:po + 64, h // 2,
                                                q * 512:(q + 1) * 512]
                                nc.scalar.copy(out=_r(ctx_sl), in_=cp[0:64, :])
                                nc.vector.tensor_tensor(
                                    out=_r(ctx_sl), in0=ctx_sl, in1=bc,
                                    op=mybir.AluOpType.mult)

                    # ---------------- proj + residual ----------------
                    with tc.tile_pool(name="wproj", bufs=8) as wproj_pool:
                        for nv in range(2):
                            w_tiles = []
                            for k in range(FC):
                                wt = wproj_pool.tile([128, 384], F32R, tag="wp")
                                nc.sync.dma_start(
                                    out=wt,
                                    in_=proj_wt_d.ap()[k * 128:(k + 1) * 128,
                                                       nv * 384:(nv + 1) * 384])
                                w_tiles.append(wt)
                            for t in range(TC):
                                ps = psum_mm.tile([128, 384], F32, tag="mm")
                                for k in range(FC):
                                    nc.tensor.matmul(
                                        ps, _r(ctx_fm[:, k, t * 128:(t + 1) * 128]),
                                        _r(w_tiles[k]),
                                        start=(k == 0), stop=(k == FC - 1))
                                sl = slice(nv * 384, (nv + 1) * 384)
                                nc.vector.tensor_add(
                                    out=x_sb[:, t, sl], in0=ps, in1=x_sb[:, t, sl])
                                nc.vector.tensor_add(
                                    out=x_sb[:, t, sl], in0=x_sb[:, t, sl],
                                    in1=projb_bc[:, sl])

            # ---------------- LN2 + transpose + MLP ----------------
            with tc.tile_pool(name="gfm", bufs=1) as g_pool:
                g_fm = g_pool.tile([128, MC_H, N_TOK], F32)
                with (
                    tc.tile_pool(name="h2fm", bufs=1) as h2fm_pool,
                    tc.tile_pool(name="h2tmp", bufs=3) as h2tmp_pool,
                    tc.tile_pool(name="wfc1", bufs=8) as wfc1_pool,
                    tc.tile_pool(name="psum_tr", bufs=2, space="PSUM") as psum_tr,
                ):
                    h2_fm = h2fm_pool.tile([128, FC, N_TOK], F32, tag="hfm")
                    for t in range(TC):
                        h_tile = h2tmp_pool.tile([128, DIM], F32, tag="h_tile")
                        _ln_chunk(nc, stat_pool, eps_tile, x_sb[:, t, :], h_tile)
                        for f in range(FC):
                            tr = psum_tr.tile([128, 128], F32, tag="tr")
                            nc.tensor.transpose(
                                tr, h_tile[:, f * 128:(f + 1) * 128], ident)
                            nc.scalar.copy(
                                out=_r(h2_fm[:, f, t * 128:(t + 1) * 128]), in_=tr)

                    for m in range(MC_H):
                        w_tiles = []
                        for k in range(FC):
                            wt = wfc1_pool.tile([128, 128], F32R, tag="wfc1")
                            nc.sync.dma_start(
                                out=wt,
                                in_=fc1_wt_d.ap()[k * 128:(k + 1) * 128,
                                                  m * 128:(m + 1) * 128])
                            w_tiles.append(wt)
                        for q in range(2):
                            ps = psum_mm.tile([128, 512], F32, tag="mm")
                            for k in range(FC):
                                nc.tensor.matmul(
                                    ps, _r(w_tiles[k]),
                                    _r(h2_fm[:, k, q * 512:(q + 1) * 512]),
                                    start=(k == 0), stop=(k == FC - 1))
                            nc.scalar.activation(
                                out=_r(g_fm[:, m, q * 512:(q + 1) * 512]), in_=ps,
                                func=mybir.ActivationFunctionType.Gelu,
                                bias=fc1b_pm[:, m:m + 1], scale=1.0)

                # fc2 + residual -> out
                with (
                    tc.tile_pool(name="wfc2", bufs=26) as wfc2_pool,
                    tc.tile_pool(name="outt", bufs=4) as out_pool,
                ):
                    for nv in range(2):
                        w_tiles = []
                        for k in range(MC_H):
                            wt = wfc2_pool.tile([128, 384], F32R, tag="wfc2")
                            nc.sync.dma_start(
                                out=wt,
                                in_=fc2_wt_d.ap()[k * 128:(k + 1) * 128,
                                                  nv * 384:(nv + 1) * 384])
                            w_tiles.append(wt)
                        for t in range(TC):
                            ps = psum_mm.tile([128, 384], F32, tag="mm")
                            for k in range(MC_H):
                                nc.tensor.matmul(
                                    ps, _r(g_fm[:, k, t * 128:(t + 1) * 128]),
                                    _r(w_tiles[k]),
                                    start=(k == 0), stop=(k == MC_H - 1))
                            sl = slice(nv * 384, (nv + 1) * 384)
                            o_t = out_pool.tile([128, 384], F32, tag="outt")
                            nc.vector.tensor_add(out=o_t, in0=ps, in1=x_sb[:, t, sl])
                            nc.vector.tensor_add(out=o_t, in0=o_t, in1=fc2b_bc[:, sl])
                            nc.sync.dma_start(out=out_dt[:, t, sl], in_=o_t)

    nc.compile()
    return nc


def host_prep(x, ln1_g, ln1_b, qkv_w, proj_w, proj_b, ln2_g, ln2_b,
              fc1_w, fc1_b, fc2_w, fc2_b):
    """Fold LN affine params into weights, pre-transpose weights."""
    f32 = np.float32
    qkv_w = np.asarray(qkv_w, f32)
    qkv_wt = np.ascontiguousarray((qkv_w * np.asarray(ln1_g, f32)[None, :]).T)
    qkv_bias = qkv_w @ np.asarray(ln1_b, f32)
    qkb_pm = np.ascontiguousarray(qkv_bias[:2 * DIM].reshape(MC_QK, 128).T)
    vb = np.ascontiguousarray(qkv_bias[2 * DIM:])
    proj_wt = np.ascontiguousarray(np.asarray(proj_w, f32).T)
    fc1_w = np.asarray(fc1_w, f32)
    fc1_wt = np.ascontiguousarray((fc1_w * np.asarray(ln2_g, f32)[None, :]).T)
    fc1_bias = fc1_w @ np.asarray(ln2_b, f32) + np.asarray(fc1_b, f32)
    fc1b_pm = np.ascontiguousarray(fc1_bias.reshape(MC_H, 128).T)
    fc2_wt = np.ascontiguousarray(np.asarray(fc2_w, f32).T)
    return {
        "qkv_wt": qkv_wt, "qkb_pm": qkb_pm, "vb": vb,
        "proj_wt": proj_wt, "projb": np.ascontiguousarray(np.asarray(proj_b, f32)),
        "fc1_wt": fc1_wt, "fc1b_pm": fc1b_pm,
        "fc2_wt": fc2_wt, "fc2b": np.ascontiguousarray(np.asarray(fc2_b, f32)),
    }


_CACHE = {}


def kernel(x, ln1_g, ln1_b, qkv_w, proj_w, proj_b, ln2_g, ln2_b,
           fc1_w, fc1_b, fc2_w, fc2_b, _want_results=False, **_ignored):
    from concourse.bass_utils import run_bass_kernel_spmd

    x = np.asarray(x, np.float32)
    B = x.shape[0]
    assert B == 8 and x.shape[1] == N_TOK and x.shape[2] == DIM

    w = host_prep(x, ln1_g, ln1_b, qkv_w, proj_w, proj_b, ln2_g, ln2_b,
                  fc1_w, fc1_b, fc2_w, fc2_b)

    if "nc" not in _CACHE:
        _CACHE["nc"] = build_bass()
    nc = _CACHE["nc"]

    in_maps = [dict(w, x=np.ascontiguousarray(x[i])) for i in range(B)]
    res = run_bass_kernel_spmd(nc, in_maps, core_ids=list(range(B)))
    out = np.stack([res.results[i]["out"] for i in range(B)], axis=0)
    if _want_results:
        return out, res
    return out


# revision 16
# speedup vs baseline: 1.3625x; 1.3625x over previous
"""Trainium2 Bass kernel for a dense transformer block (pre-LN, 12 heads, MLP 4x).

Strategy: data-parallel over batch across the 8 NeuronCores (B=8 -> one batch
element per core, no collectives). Per core:

  - residual stream kept token-major fp32 [128 tok x 768] (8 token chunks)
  - LN on DVE via bn_stats/bn_aggr, (x-mu)*rstd fused in one tensor_scalar
  - LN gains/biases folded into the following matmul weights on the host
  - matmul operands in bf16 (weights cast on host), fp32 PSUM accumulation,
    fp32 residual stream
  - h transposed to feature-major via PE transpose (48 bf16 blocks)
  - QKV: q,k produced feature-major [64d x tok per head]; v produced
    token-major directly (so P@V needs no on-chip transposes)
  - attention computed transposed: S_t[k,q] = k_fm.T @ q_fm, exp on ACT with
    fused 1/sqrt(d) scale, denominator via an extra ones-row appended to V
    (ctx psum row 64); normalization: gpsimd partition_broadcast of the
    denominator + one DVE divide during ctx evacuation
  - attention is software-pipelined: S/exp of head h+1 is emitted before
    ctx of head h so the PE never idles while ACT computes exps
  - MLP: fc1 evacuated by ACT with fused bias+GELU (exact), fc2 accumulates
    token-major, residual added during evacuation
"""

import numpy as np

import concourse.bass as bass
import concourse.mybir as mybir
import concourse.tile as tile
from concourse import bacc
from concourse.masks import make_identity

DIM = 768
HEADS = 12
HD = 64  # head dim
HIDDEN = 3072
N_TOK = 1024
TC = N_TOK // 128  # 8 token chunks
FC = DIM // 128  # 6 feature chunks
MC_QK = 2 * DIM // 128  # 12 chunks of q|k features
MC_H = HIDDEN // 128  # 24 hidden chunks
EPS = 1e-5
SCALE = HD ** -0.5

F32 = mybir.dt.float32
BF16 = mybir.dt.bfloat16


def _ln_chunk(nc, stat_pool, eps_tile, x_ap, out_ap):
    """out = (x - mean(x)) * rsqrt(var(x) + eps), row-wise over 768."""
    stats = stat_pool.tile([128, 3, 6], F32, tag="ln_stats")
    for sg in range(3):
        nc.vector.bn_stats(out=stats[:, sg, :], in_=x_ap[:, sg * 256:(sg + 1) * 256])
    mv = stat_pool.tile([128, 2], F32, tag="ln_mv")
    nc.vector.bn_aggr(out=mv, in_=stats)
    rstd = stat_pool.tile([128, 1], F32, tag="ln_rstd")
    nc.scalar.activation(
        out=rstd, in_=mv[:, 1:2], func=mybir.ActivationFunctionType.Sqrt,
        bias=eps_tile, scale=1.0,
    )
    nc.vector.reciprocal(out=rstd, in_=rstd)
    nc.vector.tensor_scalar(
        out=out_ap, in0=x_ap, scalar1=mv[:, 0:1], scalar2=rstd,
        op0=mybir.AluOpType.subtract, op1=mybir.AluOpType.mult,
    )


def build_bass():
    nc = bacc.Bacc("TRN2", debug=False)

    x_d = nc.dram_tensor("x", [N_TOK, DIM], F32, kind="ExternalInput")
    qkv_wt_d = nc.dram_tensor("qkv_wt", [DIM, 3 * DIM], BF16, kind="ExternalInput")
    qkb_pm_d = nc.dram_tensor("qkb_pm", [128, MC_QK], F32, kind="ExternalInput")
    vb_d = nc.dram_tensor("vb", [DIM], F32, kind="ExternalInput")
    proj_wt_d = nc.dram_tensor("proj_wt", [DIM, DIM], BF16, kind="ExternalInput")
    projb_d = nc.dram_tensor("projb", [DIM], F32, kind="ExternalInput")
    fc1_wt_d = nc.dram_tensor("fc1_wt", [DIM, HIDDEN], BF16, kind="ExternalInput")
    fc1b_pm_d = nc.dram_tensor("fc1b_pm", [128, MC_H], F32, kind="ExternalInput")
    fc2_wt_d = nc.dram_tensor("fc2_wt", [HIDDEN, DIM], BF16, kind="ExternalInput")
    fc2b_d = nc.dram_tensor("fc2b", [DIM], F32, kind="ExternalInput")
    out_d = nc.dram_tensor("out", [N_TOK, DIM], F32, kind="ExternalOutput")

    x_dt = x_d.ap().rearrange("(t p) c -> p t c", p=128)
    out_dt = out_d.ap().rearrange("(t p) c -> p t c", p=128)

    def bcast128(ap_1d, n):
        return bass.AP(tensor=ap_1d.tensor, offset=ap_1d.offset,
                       ap=[[0, 128], [1, n]])

    with tile.TileContext(nc) as tc:
        with (
            tc.tile_pool(name="const", bufs=1) as const_pool,
            tc.tile_pool(name="resid", bufs=1) as resid_pool,
            tc.tile_pool(name="stats", bufs=4) as stat_pool,
            tc.tile_pool(name="psum_mm", bufs=4, space="PSUM") as psum_mm,
        ):
            ident = const_pool.tile([128, 128], BF16)
            make_identity(nc, ident)
            eps_tile = const_pool.tile([128, 1], F32)
            nc.vector.memset(eps_tile, EPS)
            qkb_pm = const_pool.tile([128, MC_QK], F32)
            nc.sync.dma_start(out=qkb_pm, in_=qkb_pm_d.ap())
            fc1b_pm = const_pool.tile([128, MC_H], F32)
            nc.sync.dma_start(out=fc1b_pm, in_=fc1b_pm_d.ap())
            vb_bc = const_pool.tile([128, DIM], F32)
            nc.sync.dma_start(out=vb_bc, in_=bcast128(vb_d.ap(), DIM))
            projb_bc = const_pool.tile([128, DIM], F32)
            nc.sync.dma_start(out=projb_bc, in_=bcast128(projb_d.ap(), DIM))
            fc2b_bc = const_pool.tile([128, DIM], F32)
            nc.sync.dma_start(out=fc2b_bc, in_=bcast128(fc2b_d.ap(), DIM))
            ones_bf = const_pool.tile([128, 96], BF16)
            nc.vector.memset(ones_bf, 1.0)

            x_sb = resid_pool.tile([128, TC, DIM], F32)
            nc.sync.dma_start(out=x_sb, in_=x_dt)

            # ---------------- LN1 + transpose -> h_fm [768, 1024] -------------
            with (
                tc.tile_pool(name="qk", bufs=1) as qk_pool,
                tc.tile_pool(name="vaug", bufs=1) as v_pool,
            ):
                qk_fm = qk_pool.tile([128, MC_QK, N_TOK], BF16)
                v_aug = v_pool.tile([128, TC, HEADS, HD + 1], BF16)

                with (
                    tc.tile_pool(name="hfm", bufs=1) as hfm_pool,
                    tc.tile_pool(name="htmp", bufs=3) as htmp_pool,
                    tc.tile_pool(name="wqkv", bufs=14) as wqkv_pool,
                    tc.tile_pool(name="wv", bufs=8) as wv_pool,
                    tc.tile_pool(name="psum_tr", bufs=2, space="PSUM") as psum_tr,
                ):
                    h_fm = hfm_pool.tile([128, FC, N_TOK], BF16, tag="hfm")
                    for t in range(TC):
                        h_tile = htmp_pool.tile([128, DIM], BF16, tag="h_tile")
                        _ln_chunk(nc, stat_pool, eps_tile, x_sb[:, t, :], h_tile)
                        for f in range(FC):
                            tr = psum_tr.tile([128, 128], BF16, tag="tr")
                            nc.tensor.transpose(
                                tr, h_tile[:, f * 128:(f + 1) * 128], ident)
                            nc.scalar.copy(
                                out=h_fm[:, f, t * 128:(t + 1) * 128], in_=tr)

                    # ---------------- QKV ----------------
                    # v token-major with ones column: v_aug [128, 8, 12, 65]
                    nc.scalar.copy(
                        out=v_aug[:, :, :, HD],
                        in_=ones_bf[:, 0:96].rearrange("p (a b) -> p a b", a=TC))
                    for nv in range(2):
                        w_tiles = []
                        for k in range(FC):
                            wt = wv_pool.tile([128, 384], BF16, tag="wv")
                            nc.sync.dma_start(
                                out=wt,
                                in_=qkv_wt_d.ap()[k * 128:(k + 1) * 128,
                                                  2 * DIM + nv * 384:
                                                  2 * DIM + (nv + 1) * 384])
                            w_tiles.append(wt)
                        for t in range(TC):
                            ps = psum_mm.tile([128, 384], F32, tag="mm")
                            for k in range(FC):
                                nc.tensor.matmul(
                                    ps, h_fm[:, k, t * 128:(t + 1) * 128],
                                    w_tiles[k],
                                    start=(k == 0), stop=(k == FC - 1))
                            nc.vector.tensor_add(
                                out=v_aug[:, t, nv * 6:(nv + 1) * 6, 0:HD],
                                in0=ps.rearrange("p (h d) -> p h d", d=HD),
                                in1=vb_bc[:, nv * 384:(nv + 1) * 384].rearrange(
                                    "p (h d) -> p h d", d=HD))

                    # q,k feature-major: qk_fm [128, 12, 1024]; emit the q and
                    # k chunks of each head pair together so attention can
                    # start as soon as possible
                    for m in [0, 6, 1, 7, 2, 8, 3, 9, 4, 10, 5, 11]:
                        w_tiles = []
                        for k in range(FC):
                            wt = wqkv_pool.tile([128, 128], BF16, tag="wqkv")
                            nc.sync.dma_start(
                                out=wt,
                                in_=qkv_wt_d.ap()[k * 128:(k + 1) * 128,
                                                  m * 128:(m + 1) * 128])
                            w_tiles.append(wt)
                        for q in range(2):
                            ps = psum_mm.tile([128, 512], F32, tag="mm")
                            for k in range(FC):
                                nc.tensor.matmul(
                                    ps, w_tiles[k],
                                    h_fm[:, k, q * 512:(q + 1) * 512],
                                    start=(k == 0), stop=(k == FC - 1))
                            nc.vector.tensor_scalar_add(
                                out=qk_fm[:, m, q * 512:(q + 1) * 512], in0=ps,
                                scalar1=qkb_pm[:, m:m + 1])

                # ---------------- attention (software-pipelined) -----------
                with tc.tile_pool(name="ctxfm", bufs=1) as ctx_pool:
                    ctx_fm = ctx_pool.tile([128, FC, N_TOK], BF16)
                    with (
                        tc.tile_pool(name="exps", bufs=18) as exps_pool,
                        tc.tile_pool(name="dsmall", bufs=4) as dsmall_pool,
                        tc.tile_pool(name="psum_ctx", bufs=3, space="PSUM")
                        as psum_ctx,
                    ):
                        def emit_ctx(h, exps):
                            po = 64 * (h % 2)
                            for q in range(2):
                                cp = psum_ctx.tile([65, 512], F32, tag="ctx")
                                for kc in range(TC):
                                    nc.tensor.matmul(
                                        cp, v_aug[:, kc, h, :],
                                        exps[kc][:, q * 512:(q + 1) * 512],
                                        start=(kc == 0), stop=(kc == TC - 1))
                                den = dsmall_pool.tile([1, 512], F32, tag="den")
                                nc.scalar.copy(out=den, in_=cp[64:65, :])
                                scr = dsmall_pool.tile([1, 512], F32, tag="scr")
                                rec = dsmall_pool.tile([1, 512], F32, tag="rec")
                                nc.vector.reciprocal_approx_accurate(
                                    out=rec, in_=den, scratch=scr)
                                bcd = dsmall_pool.tile([128, 512], F32, tag="bcd")
                                nc.gpsimd.partition_broadcast(bcd, rec)
                                ctx_sl = ctx_fm[po:po + 64, h // 2,
                                                q * 512:(q + 1) * 512]
                                nc.scalar.copy(out=ctx_sl, in_=cp[0:64, :])
                                nc.vector.tensor_tensor(
                                    out=ctx_sl, in0=ctx_sl,
                                    in1=bcd[po:po + 64, :],
                                    op=mybir.AluOpType.mult)

                        pend = None
                        for h in range(HEADS):
                            po = 64 * (h % 2)
                            q_fm_h = qk_fm[po:po + 64, h // 2, :]
                            k_fm_h = qk_fm[po:po + 64, 6 + h // 2, :]
                            exps = []
                            for kc in range(TC):
                                e_t = exps_pool.tile([128, N_TOK], BF16,
                                                     tag="exps")
                                for q in range(2):
                                    sp = psum_mm.tile([128, 512], F32, tag="mm")
                                    nc.tensor.matmul(
                                        sp, k_fm_h[:, kc * 128:(kc + 1) * 128],
                                        q_fm_h[:, q * 512:(q + 1) * 512],
                                        start=True, stop=True)
                                    nc.scalar.activation(
                                        out=e_t[:, q * 512:(q + 1) * 512],
                                        in_=sp,
                                        func=mybir.ActivationFunctionType.Exp,
                                        scale=SCALE)
                                exps.append(e_t)
                            if pend is not None:
                                emit_ctx(*pend)
                            pend = (h, exps)
                        emit_ctx(*pend)

                    # ---------------- proj + residual ----------------
                    with tc.tile_pool(name="wproj", bufs=8) as wproj_pool:
                        for nv in range(2):
                            w_tiles = []
                            for k in range(FC):
                                wt = wproj_pool.tile([128, 384], BF16, tag="wp")
                                nc.sync.dma_start(
                                    out=wt,
                                    in_=proj_wt_d.ap()[k * 128:(k + 1) * 128,
                                                       nv * 384:(nv + 1) * 384])
                                w_tiles.append(wt)
                            for t in range(TC):
                                ps = psum_mm.tile([128, 384], F32, tag="mm")
                                for k in range(FC):
                                    nc.tensor.matmul(
                                        ps, ctx_fm[:, k, t * 128:(t + 1) * 128],
                                        w_tiles[k],
                                        start=(k == 0), stop=(k == FC - 1))
                                sl = slice(nv * 384, (nv + 1) * 384)
                                nc.vector.tensor_add(
                                    out=x_sb[:, t, sl], in0=ps, in1=x_sb[:, t, sl])
                                nc.vector.tensor_add(
                                    out=x_sb[:, t, sl], in0=x_sb[:, t, sl],
                                    in1=projb_bc[:, sl])

            # ---------------- LN2 + transpose + MLP ----------------
            with tc.tile_pool(name="gfm", bufs=1) as g_pool:
                g_fm = g_pool.tile([128, MC_H, N_TOK], BF16)
                with (
                    tc.tile_pool(name="h2fm", bufs=1) as h2fm_pool,
                    tc.tile_pool(name="h2tmp", bufs=3) as h2tmp_pool,
                    tc.tile_pool(name="wfc1", bufs=14) as wfc1_pool,
                    tc.tile_pool(name="psum_tr", bufs=2, space="PSUM") as psum_tr,
                ):
                    h2_fm = h2fm_pool.tile([128, FC, N_TOK], BF16, tag="hfm")
                    for t in range(TC):
                        h_tile = h2tmp_pool.tile([128, DIM], BF16, tag="h_tile")
                        _ln_chunk(nc, stat_pool, eps_tile, x_sb[:, t, :], h_tile)
                        for f in range(FC):
                            tr = psum_tr.tile([128, 128], BF16, tag="tr")
                            nc.tensor.transpose(
                                tr, h_tile[:, f * 128:(f + 1) * 128], ident)
                            nc.scalar.copy(
                                out=h2_fm[:, f, t * 128:(t + 1) * 128], in_=tr)

                    for m in range(MC_H):
                        w_tiles = []
                        for k in range(FC):
                            wt = wfc1_pool.tile([128, 128], BF16, tag="wfc1")
                            nc.sync.dma_start(
                                out=wt,
                                in_=fc1_wt_d.ap()[k * 128:(k + 1) * 128,
                                                  m * 128:(m + 1) * 128])
                            w_tiles.append(wt)
                        for q in range(2):
                            ps = psum_mm.tile([128, 512], F32, tag="mm")
                            for k in range(FC):
                                nc.tensor.matmul(
                                    ps, w_tiles[k],
                                    h2_fm[:, k, q * 512:(q + 1) * 512],
                                    start=(k == 0), stop=(k == FC - 1))
                            nc.scalar.activation(
                                out=g_fm[:, m, q * 512:(q + 1) * 512], in_=ps,
                                func=mybir.ActivationFunctionType.Gelu,
                                bias=fc1b_pm[:, m:m + 1], scale=1.0)

                # fc2 + residual -> out
                with (
                    tc.tile_pool(name="wfc2", bufs=26) as wfc2_pool,
                    tc.tile_pool(name="outt", bufs=4) as out_pool,
                ):
                    for nv in range(2):
                        w_tiles = []
                        for k in range(MC_H):
                            wt = wfc2_pool.tile([128, 384], BF16, tag="wfc2")
                            nc.sync.dma_start(
                                out=wt,
                                in_=fc2_wt_d.ap()[k * 128:(k + 1) * 128,
                                                  nv * 384:(nv + 1) * 384])
                            w_tiles.append(wt)
                        for t in range(TC):
                            ps = psum_mm.tile([128, 384], F32, tag="mm")
                            for k in range(MC_H):
                                nc.tensor.matmul(
                                    ps, g_fm[:, k, t * 128:(t + 1) * 128],
                                    w_tiles[k],
                                    start=(k == 0), stop=(k == MC_H - 1))
                            sl = slice(nv * 384, (nv + 1) * 384)
                            o_t = out_pool.tile([128, 384], F32, tag="outt")
                            nc.vector.tensor_add(out=o_t, in0=ps, in1=x_sb[:, t, sl])
                            nc.vector.tensor_add(out=o_t, in0=o_t, in1=fc2b_bc[:, sl])
                            nc.sync.dma_start(out=out_dt[:, t, sl], in_=o_t)

    nc.compile()
    return nc


def host_prep(x, ln1_g, ln1_b, qkv_w, proj_w, proj_b, ln2_g, ln2_b,
              fc1_w, fc1_b, fc2_w, fc2_b):
    """Fold LN affine params into weights, pre-transpose, cast to bf16."""
    import ml_dtypes
    f32 = np.float32
    bf16 = ml_dtypes.bfloat16
    qkv_w = np.asarray(qkv_w, f32)
    qkv_wt = np.ascontiguousarray(
        (qkv_w * np.asarray(ln1_g, f32)[None, :]).T).astype(bf16)
    qkv_bias = qkv_w @ np.asarray(ln1_b, f32)
    qkb_pm = np.ascontiguousarray(qkv_bias[:2 * DIM].reshape(MC_QK, 128).T)
    vb = np.ascontiguousarray(qkv_bias[2 * DIM:])
    proj_wt = np.ascontiguousarray(np.asarray(proj_w, f32).T).astype(bf16)
    fc1_w = np.asarray(fc1_w, f32)
    fc1_wt = np.ascontiguousarray(
        (fc1_w * np.asarray(ln2_g, f32)[None, :]).T).astype(bf16)
    fc1_bias = fc1_w @ np.asarray(ln2_b, f32) + np.asarray(fc1_b, f32)
    fc1b_pm = np.ascontiguousarray(fc1_bias.reshape(MC_H, 128).T)
    fc2_wt = np.ascontiguousarray(np.asarray(fc2_w, f32).T).astype(bf16)
    return {
        "qkv_wt": qkv_wt, "qkb_pm": qkb_pm, "vb": vb,
        "proj_wt": proj_wt, "projb": np.ascontiguousarray(np.asarray(proj_b, f32)),
        "fc1_wt": fc1_wt, "fc1b_pm": fc1b_pm,
        "fc2_wt": fc2_wt, "fc2b": np.ascontiguousarray(np.asarray(fc2_b, f32)),
    }


_CACHE = {}


def kernel(x, ln1_g, ln1_b, qkv_w, proj_w, proj_b, ln2_g, ln2_b,
           fc1_w, fc1_b, fc2_w, fc2_b, _want_results=False, **_ignored):
    from concourse.bass_utils import run_bass_kernel_spmd

    x = np.asarray(x, np.float32)
    B = x.shape[0]
    assert B == 8 and x.shape[1] == N_TOK and x.shape[2] == DIM

    w = host_prep(x, ln1_g, ln1_b, qkv_w, proj_w, proj_b, ln2_g, ln2_b,
                  fc1_w, fc1_b, fc2_w, fc2_b)

    if "nc" not in _CACHE:
        _CACHE["nc"] = build_bass()
    nc = _CACHE["nc"]

    in_maps = [dict(w, x=np.ascontiguousarray(x[i])) for i in range(B)]
    res = run_bass_kernel_spmd(nc, in_maps, core_ids=list(range(B)))
    out = np.stack([res.results[i]["out"] for i in range(B)], axis=0)
    if _want_results:
        return out, res
    return out


# revision 23
# speedup vs baseline: 1.6448x; 1.2072x over previous
"""Trainium2 Bass kernel for a dense transformer block (pre-LN, 12 heads, MLP 4x).

Strategy: data-parallel over batch across the 8 NeuronCores (B=8 -> one batch
element per core, no collectives). Per core:

  - residual stream kept token-major fp32 [128 tok x 768] (8 token chunks)
  - LN on DVE via bn_stats/bn_aggr; LN affine params folded into the weights
    on the host; matmul operands bf16, PSUM accumulation fp32
  - h transposed to feature-major via PE transposes batched 6-to-a-PSUM-bank
  - QKV: q,k produced feature-major [64d x tok per head]; v produced
    token-major directly (no on-chip transposes for v); one 3D-AP DMA per
    weight chunk group
  - attention computed transposed: S_t[k,q] = k_fm.T @ q_fm with the two heads
    of a pair packed into disjoint PE row groups (base partitions 0/64) so
    their matmuls run concurrently; exp on ACT over [128,1024] PSUM tiles with
    fused 1/sqrt(d); softmax denominator from a ones-row appended to V; the
    normalization is gpsimd partition_broadcast + one DVE mult straight out
    of PSUM. S/exp for a pair is emitted inside the QKV loop right after its
    q/k chunks, and ctx of the previous pair follows, so PE and ACT overlap
    across the whole attention region.
  - MLP: fc1 -> ACT gelu (exact, fused bias) -> fc2, with LN2 interleaved
    into the proj loop
"""

import numpy as np

import concourse.bass as bass
import concourse.mybir as mybir
import concourse.tile as tile
from concourse import bacc
from concourse.masks import make_identity

DIM = 768
HEADS = 12
HD = 64  # head dim
HIDDEN = 3072
N_TOK = 1024
TC = N_TOK // 128  # 8 token chunks
FC = DIM // 128  # 6 feature chunks
MC_QK = 2 * DIM // 128  # 12 chunks of q|k features
MC_H = HIDDEN // 128  # 24 hidden chunks
EPS = 1e-5
SCALE = HD ** -0.5

F32 = mybir.dt.float32
BF16 = mybir.dt.bfloat16


def _ln_chunk(nc, stat_pool, eps_tile, x_ap, out_ap):
    """out = (x - mean(x)) * rsqrt(var(x) + eps), row-wise over 768."""
    stats = stat_pool.tile([128, 3, 6], F32, tag="ln_stats")
    for sg in range(3):
        nc.vector.bn_stats(out=stats[:, sg, :], in_=x_ap[:, sg * 256:(sg + 1) * 256])
    mv = stat_pool.tile([128, 2], F32, tag="ln_mv")
    nc.vector.bn_aggr(out=mv, in_=stats)
    rstd = stat_pool.tile([128, 1], F32, tag="ln_rstd")
    nc.scalar.activation(
        out=rstd, in_=mv[:, 1:2], func=mybir.ActivationFunctionType.Sqrt,
        bias=eps_tile, scale=1.0,
    )
    nc.vector.reciprocal(out=rstd, in_=rstd)
    nc.vector.tensor_scalar(
        out=out_ap, in0=x_ap, scalar1=mv[:, 0:1], scalar2=rstd,
        op0=mybir.AluOpType.subtract, op1=mybir.AluOpType.mult,
    )


def build_bass():
    nc = bacc.Bacc("TRN2", debug=False)

    x_d = nc.dram_tensor("x", [N_TOK, DIM], F32, kind="ExternalInput")
    qkv_wt_d = nc.dram_tensor("qkv_wt", [DIM, 3 * DIM], BF16, kind="ExternalInput")
    qkb_pm_d = nc.dram_tensor("qkb_pm", [128, MC_QK], F32, kind="ExternalInput")
    vb_d = nc.dram_tensor("vb", [DIM], F32, kind="ExternalInput")
    proj_wt_d = nc.dram_tensor("proj_wt", [DIM, DIM], BF16, kind="ExternalInput")
    projb_d = nc.dram_tensor("projb", [DIM], F32, kind="ExternalInput")
    fc1_wt_d = nc.dram_tensor("fc1_wt", [DIM, HIDDEN], BF16, kind="ExternalInput")
    fc1b_pm_d = nc.dram_tensor("fc1b_pm", [128, MC_H], F32, kind="ExternalInput")
    fc2_wt_d = nc.dram_tensor("fc2_wt", [HIDDEN, DIM], BF16, kind="ExternalInput")
    fc2b_d = nc.dram_tensor("fc2b", [DIM], F32, kind="ExternalInput")
    out_d = nc.dram_tensor("out", [N_TOK, DIM], F32, kind="ExternalOutput")

    x_dt = x_d.ap().rearrange("(t p) c -> p t c", p=128)
    out_dt = out_d.ap().rearrange("(t p) c -> p t c", p=128)
    # weight chunk views: [128 part of in-feat, in-chunk, out-col]
    qkv_w3 = qkv_wt_d.ap().rearrange("(ko p) n -> p ko n", p=128)
    proj_w3 = proj_wt_d.ap().rearrange("(ko p) n -> p ko n", p=128)
    fc1_w3 = fc1_wt_d.ap().rearrange("(ko p) n -> p ko n", p=128)
    fc2_w3 = fc2_wt_d.ap().rearrange("(ko p) n -> p ko n", p=128)

    def bcast128(ap_1d, n):
        return bass.AP(tensor=ap_1d.tensor, offset=ap_1d.offset,
                       ap=[[0, 128], [1, n]])

    with tile.TileContext(nc) as tc:
        with (
            tc.tile_pool(name="const", bufs=1) as const_pool,
            tc.tile_pool(name="resid", bufs=1) as resid_pool,
            tc.tile_pool(name="stats", bufs=4) as stat_pool,
            # PSUM: big 2-bank tiles (S pairs, fc1, batched transposes),
            # small 1-bank tiles (qkv/v/proj/fc2), ctx tiles. 4+2+2 = 8 banks.
            tc.tile_pool(name="psum_big", bufs=2, space="PSUM") as psum_big,
            tc.tile_pool(name="psum_small", bufs=2, space="PSUM") as psum_small,
            tc.tile_pool(name="psum_ctx", bufs=2, space="PSUM") as psum_ctx,
            tc.tile_pool(name="h2fm", bufs=1) as h2fm_pool,
        ):
            ident = const_pool.tile([128, 128], BF16)
            make_identity(nc, ident)
            eps_tile = const_pool.tile([128, 1], F32)
            nc.vector.memset(eps_tile, EPS)
            qkb_pm = const_pool.tile([128, MC_QK], F32)
            nc.sync.dma_start(out=qkb_pm, in_=qkb_pm_d.ap())
            fc1b_pm = const_pool.tile([128, MC_H], F32)
            nc.sync.dma_start(out=fc1b_pm, in_=fc1b_pm_d.ap())
            vb_bc = const_pool.tile([128, DIM], F32)
            nc.sync.dma_start(out=vb_bc, in_=bcast128(vb_d.ap(), DIM))
            projb_bc = const_pool.tile([128, DIM], F32)
            nc.sync.dma_start(out=projb_bc, in_=bcast128(projb_d.ap(), DIM))
            fc2b_bc = const_pool.tile([128, DIM], F32)
            nc.sync.dma_start(out=fc2b_bc, in_=bcast128(fc2b_d.ap(), DIM))
            ones_bf = const_pool.tile([128, 96], BF16)
            nc.vector.memset(ones_bf, 1.0)

            x_sb = resid_pool.tile([128, TC, DIM], F32)
            h2_fm = h2fm_pool.tile([128, FC, N_TOK], BF16, tag="hfm2")

            def ln_transpose(t, dst_fm):
                """LN of token chunk t + PE-transpose into dst_fm[:, :, t*128:]."""
                h_tile = stat_pool.tile([128, DIM], BF16, tag="h_tile")
                _ln_chunk(nc, stat_pool, eps_tile, x_sb[:, t, :], h_tile)
                tr = psum_big.tile([128, FC, 128], BF16, tag="big")
                for f in range(FC):
                    nc.tensor.transpose(
                        tr[:, f, :], h_tile[:, f * 128:(f + 1) * 128], ident)
                nc.scalar.copy(
                    out=dst_fm[:, :, t * 128:(t + 1) * 128], in_=tr)

            # ============ attention region (qkv + attention + proj) =========
            with (
                tc.tile_pool(name="qk", bufs=1) as qk_pool,
                tc.tile_pool(name="vaug", bufs=1) as v_pool,
                tc.tile_pool(name="ctxfm", bufs=1) as ctx_pool,
            ):
                qk_fm = qk_pool.tile([128, MC_QK, N_TOK], BF16)
                v_aug = v_pool.tile([128, TC, HEADS, HD + 1], BF16)
                ctx_fm = ctx_pool.tile([128, FC, N_TOK], BF16)

                with (
                    tc.tile_pool(name="hfm", bufs=1) as hfm_pool,
                    tc.tile_pool(name="wqkv", bufs=3) as wqkv_pool,
                    tc.tile_pool(name="wvp", bufs=2) as wv_pool,
                    tc.tile_pool(name="exps", bufs=26) as exps_pool,
                    tc.tile_pool(name="dsmall", bufs=2) as dsmall_pool,
                ):
                    h_fm = hfm_pool.tile([128, FC, N_TOK], BF16, tag="hfm")

                    # v weights resident: [128, 6, 384] x2
                    wv = [wv_pool.tile([128, FC, 384], BF16, tag="wv",
                                       name=f"wv{i}") for i in range(2)]
                    for nv in range(2):
                        nc.sync.dma_start(
                            out=wv[nv],
                            in_=qkv_w3[:, :, 2 * DIM + nv * 384:
                                       2 * DIM + (nv + 1) * 384])
                    nc.scalar.copy(
                        out=v_aug[:, :, :, HD],
                        in_=ones_bf[:, 0:96].rearrange("p (a b) -> p a b", a=TC))

                    # LN1 + v per token chunk (v starts the PE early)
                    for t in range(TC):
                        nc.sync.dma_start(out=x_sb[:, t, :], in_=x_dt[:, t, :])
                        ln_transpose(t, h_fm)
                        for nv in range(2):
                            ps = psum_small.tile([128, 384], F32, tag="sm")
                            for k in range(FC):
                                nc.tensor.matmul(
                                    ps, h_fm[:, k, t * 128:(t + 1) * 128],
                                    wv[nv][:, k, :],
                                    start=(k == 0), stop=(k == FC - 1))
                            nc.vector.tensor_add(
                                out=v_aug[:, t, nv * 6:(nv + 1) * 6, 0:HD],
                                in0=ps.rearrange("p (h d) -> p h d", d=HD),
                                in1=vb_bc[:, nv * 384:(nv + 1) * 384].rearrange(
                                    "p (h d) -> p h d", d=HD))

                    def emit_qk(m):
                        wt = wqkv_pool.tile([128, FC, 128], BF16, tag="wqkv")
                        nc.sync.dma_start(
                            out=wt, in_=qkv_w3[:, :, m * 128:(m + 1) * 128])
                        for q in range(2):
                            ps = psum_small.tile([128, 512], F32, tag="sm")
                            for k in range(FC):
                                nc.tensor.matmul(
                                    ps, wt[:, k, :],
                                    h_fm[:, k, q * 512:(q + 1) * 512],
                                    start=(k == 0), stop=(k == FC - 1))
                            nc.vector.tensor_scalar_add(
                                out=qk_fm[:, m, q * 512:(q + 1) * 512], in0=ps,
                                scalar1=qkb_pm[:, m:m + 1])

                    def emit_s_exp(j):
                        """S + exp for head pair (2j, 2j+1); returns exps."""
                        exps = []  # [kc][ab]
                        for kc in range(TC):
                            sps = []
                            for ab in range(2):
                                po = 64 * ab
                                sp = psum_big.tile([128, N_TOK], F32, tag="big")
                                for q in range(2):
                                    nc.tensor.matmul(
                                        sp[:, q * 512:(q + 1) * 512],
                                        qk_fm[po:po + 64, 6 + j,
                                              kc * 128:(kc + 1) * 128],
                                        qk_fm[po:po + 64, j,
                                              q * 512:(q + 1) * 512],
                                        start=True, stop=True)
                                sps.append(sp)
                            pair = []
                            for ab in range(2):
                                e_t = exps_pool.tile([128, N_TOK], BF16,
                                                     tag="exps")
                                nc.scalar.activation(
                                    out=e_t, in_=sps[ab],
                                    func=mybir.ActivationFunctionType.Exp,
                                    scale=SCALE)
                                pair.append(e_t)
                            exps.append(pair)
                        return exps

                    def emit_ctx(j, exps):
                        for ab in range(2):
                            h = 2 * j + ab
                            po = 64 * ab
                            for q in range(2):
                                cp = psum_ctx.tile([65, 512], F32, tag="ctx")
                                for kc in range(TC):
                                    nc.tensor.matmul(
                                        cp, v_aug[:, kc, h, :],
                                        exps[kc][ab][:, q * 512:(q + 1) * 512],
                                        start=(kc == 0), stop=(kc == TC - 1))
                                den = dsmall_pool.tile([1, 512], F32, tag="den")
                                nc.scalar.copy(out=den, in_=cp[64:65, :])
                                scr = dsmall_pool.tile([1, 512], F32, tag="scr")
                                rec = dsmall_pool.tile([1, 512], F32, tag="rec")
                                nc.vector.reciprocal_approx_accurate(
                                    out=rec, in_=den, scratch=scr)
                                bcd = dsmall_pool.tile([128, 512], F32,
                                                       tag="bcd")
                                nc.gpsimd.partition_broadcast(bcd, rec)
                                nc.vector.tensor_tensor(
                                    out=ctx_fm[po:po + 64, h // 2,
                                               q * 512:(q + 1) * 512],
                                    in0=cp[0:64, :], in1=bcd[po:po + 64, :],
                                    op=mybir.AluOpType.mult)

                    pend = None
                    for j in range(6):
                        emit_qk(j)
                        emit_qk(6 + j)
                        exps = emit_s_exp(j)
                        if pend is not None:
                            emit_ctx(*pend)
                        pend = (j, exps)
                    emit_ctx(*pend)

                # ---------------- proj + residual + LN2 ----------------
                with tc.tile_pool(name="wproj", bufs=2) as wproj_pool:
                    wp = [wproj_pool.tile([128, FC, 384], BF16, tag="wp",
                                          name=f"wp{i}") for i in range(2)]
                    for nv in range(2):
                        nc.sync.dma_start(
                            out=wp[nv],
                            in_=proj_w3[:, :, nv * 384:(nv + 1) * 384])
                    for t in range(TC):
                        for nv in range(2):
                            ps = psum_small.tile([128, 384], F32, tag="sm")
                            for k in range(FC):
                                nc.tensor.matmul(
                                    ps, ctx_fm[:, k, t * 128:(t + 1) * 128],
                                    wp[nv][:, k, :],
                                    start=(k == 0), stop=(k == FC - 1))
                            sl = slice(nv * 384, (nv + 1) * 384)
                            nc.vector.tensor_add(
                                out=x_sb[:, t, sl], in0=ps, in1=x_sb[:, t, sl])
                            nc.vector.tensor_add(
                                out=x_sb[:, t, sl], in0=x_sb[:, t, sl],
                                in1=projb_bc[:, sl])
                        ln_transpose(t, h2_fm)

            # ---------------- MLP: fc1 / gelu / fc2 ----------------
            with (
                tc.tile_pool(name="gfm", bufs=1) as g_pool,
                tc.tile_pool(name="wfc1", bufs=1) as wfc1_pool,
                tc.tile_pool(name="wfc2", bufs=2) as wfc2_pool,
                tc.tile_pool(name="outt", bufs=4) as out_pool,
            ):
                g_fm = g_pool.tile([128, MC_H, N_TOK], BF16)
                wf1 = wfc1_pool.tile([128, FC, HIDDEN], BF16)
                nc.sync.dma_start(out=wf1, in_=fc1_w3)
                wf2 = [wfc2_pool.tile([128, MC_H, 384], BF16,
                                      tag="wf2", name=f"wf2{i}")
                       for i in range(2)]
                for nv in range(2):
                    nc.sync.dma_start(
                        out=wf2[nv],
                        in_=fc2_w3[:, :, nv * 384:(nv + 1) * 384])

                for m in range(MC_H):
                    ps = psum_big.tile([128, N_TOK], F32, tag="big")
                    for half in range(2):
                        for k in range(FC):
                            nc.tensor.matmul(
                                ps[:, half * 512:(half + 1) * 512],
                                wf1[:, k, m * 128:(m + 1) * 128],
                                h2_fm[:, k, half * 512:(half + 1) * 512],
                                start=(k == 0), stop=(k == FC - 1))
                    nc.scalar.activation(
                        out=g_fm[:, m, :], in_=ps,
                        func=mybir.ActivationFunctionType.Gelu,
                        bias=fc1b_pm[:, m:m + 1], scale=1.0)

                for t in range(TC):
                    for nv in range(2):
                        ps = psum_small.tile([128, 384], F32, tag="sm")
                        for k in range(MC_H):
                            nc.tensor.matmul(
                                ps, g_fm[:, k, t * 128:(t + 1) * 128],
                                wf2[nv][:, k, :],
                                start=(k == 0), stop=(k == MC_H - 1))
                        sl = slice(nv * 384, (nv + 1) * 384)
                        o_t = out_pool.tile([128, 384], F32, tag="outt")
                        nc.vector.tensor_add(out=o_t, in0=ps, in1=x_sb[:, t, sl])
                        nc.vector.tensor_add(out=o_t, in0=o_t, in1=fc2b_bc[:, sl])
                        nc.sync.dma_start(out=out_dt[:, t, sl], in_=o_t)

    nc.compile()
    return nc


def host_prep(x, ln1_g, ln1_b, qkv_w, proj_w, proj_b, ln2_g, ln2_b,
              fc1_w, fc1_b, fc2_w, fc2_b):
    """Fold LN affine params into weights, pre-transpose, cast to bf16."""
    import ml_dtypes
    f32 = np.float32
    bf16 = ml_dtypes.bfloat16
    qkv_w = np.asarray(qkv_w, f32)
    qkv_wt = np.ascontiguousarray(
        (qkv_w * np.asarray(ln1_g, f32)[None, :]).T).astype(bf16)
    qkv_bias = qkv_w @ np.asarray(ln1_b, f32)
    qkb_pm = np.ascontiguousarray(qkv_bias[:2 * DIM].reshape(MC_QK, 128).T)
    vb = np.ascontiguousarray(qkv_bias[2 * DIM:])
    proj_wt = np.ascontiguousarray(np.asarray(proj_w, f32).T).astype(bf16)
    fc1_w = np.asarray(fc1_w, f32)
    fc1_wt = np.ascontiguousarray(
        (fc1_w * np.asarray(ln2_g, f32)[None, :]).T).astype(bf16)
    fc1_bias = fc1_w @ np.asarray(ln2_b, f32) + np.asarray(fc1_b, f32)
    fc1b_pm = np.ascontiguousarray(fc1_bias.reshape(MC_H, 128).T)
    fc2_wt = np.ascontiguousarray(np.asarray(fc2_w, f32).T).astype(bf16)
    return {
        "qkv_wt": qkv_wt, "qkb_pm": qkb_pm, "vb": vb,
        "proj_wt": proj_wt, "projb": np.ascontiguousarray(np.asarray(proj_b, f32)),
        "fc1_wt": fc1_wt, "fc1b_pm": fc1b_pm,
        "fc2_wt": fc2_wt, "fc2b": np.ascontiguousarray(np.asarray(fc2_b, f32)),
    }


_CACHE = {}


def kernel(x, ln1_g, ln1_b, qkv_w, proj_w, proj_b, ln2_g, ln2_b,
           fc1_w, fc1_b, fc2_w, fc2_b, _want_results=False, **_ignored):
    from concourse.bass_utils import run_bass_kernel_spmd

    x = np.asarray(x, np.float32)
    B = x.shape[0]
    assert B == 8 and x.shape[1] == N_TOK and x.shape[2] == DIM

    w = host_prep(x, ln1_g, ln1_b, qkv_w, proj_w, proj_b, ln2_g, ln2_b,
                  fc1_w, fc1_b, fc2_w, fc2_b)

    if "nc" not in _CACHE:
        _CACHE["nc"] = build_bass()
    nc = _CACHE["nc"]

    in_maps = [dict(w, x=np.ascontiguousarray(x[i])) for i in range(B)]
    res = run_bass_kernel_spmd(nc, in_maps, core_ids=list(range(B)))
    out = np.stack([res.results[i]["out"] for i in range(B)], axis=0)
    if _want_results:
        return out, res
    return out


# revision 26
# speedup vs baseline: 1.6688x; 1.0146x over previous
"""Trainium2 Bass kernel for a dense transformer block (pre-LN, 12 heads, MLP 4x).

Strategy: data-parallel over batch across the 8 NeuronCores (B=8 -> one batch
element per core, no collectives). Per core:

  - residual stream kept token-major fp32 [128 tok x 768] (8 token chunks)
  - LN on DVE via bn_stats/bn_aggr; LN affine params folded into the weights
    on the host; matmul operands bf16, PSUM accumulation fp32
  - h transposed to feature-major via PE transposes batched 6-to-a-PSUM-bank
  - QKV: q,k produced feature-major [64d x tok per head]; v produced
    token-major directly (no on-chip transposes for v); one 3D-AP DMA per
    weight chunk group
  - attention computed transposed: S_t[k,q] = k_fm.T @ q_fm with the two heads
    of a pair packed into disjoint PE row groups (base partitions 0/64) so
    their matmuls run concurrently; exp on ACT over [128,1024] PSUM tiles with
    fused 1/sqrt(d); softmax denominator from a ones-row appended to V; the
    normalization is gpsimd partition_broadcast + one DVE mult straight out
    of PSUM. S/exp for a pair is emitted inside the QKV loop right after its
    q/k chunks, and ctx of the previous pair follows, so PE and ACT overlap
    across the whole attention region.
  - MLP: fc1 -> ACT gelu (exact, fused bias) -> fc2, with LN2 interleaved
    into the proj loop
"""

import numpy as np

import concourse.bass as bass
import concourse.mybir as mybir
import concourse.tile as tile
from concourse import bacc
from concourse.masks import make_identity

DIM = 768
HEADS = 12
HD = 64  # head dim
HIDDEN = 3072
N_TOK = 1024
TC = N_TOK // 128  # 8 token chunks
FC = DIM // 128  # 6 feature chunks
MC_QK = 2 * DIM // 128  # 12 chunks of q|k features
MC_H = HIDDEN // 128  # 24 hidden chunks
EPS = 1e-5
SCALE = HD ** -0.5

F32 = mybir.dt.float32
BF16 = mybir.dt.bfloat16


def _ln_chunk(nc, stat_pool, eps_tile, x_ap, out_ap):
    """out = (x - mean(x)) * rsqrt(var(x) + eps), row-wise over 768."""
    stats = stat_pool.tile([128, 3, 6], F32, tag="ln_stats")
    for sg in range(3):
        nc.vector.bn_stats(out=stats[:, sg, :], in_=x_ap[:, sg * 256:(sg + 1) * 256])
    mv = stat_pool.tile([128, 2], F32, tag="ln_mv")
    nc.vector.bn_aggr(out=mv, in_=stats)
    rstd = stat_pool.tile([128, 1], F32, tag="ln_rstd")
    nc.scalar.activation(
        out=rstd, in_=mv[:, 1:2], func=mybir.ActivationFunctionType.Sqrt,
        bias=eps_tile, scale=1.0,
    )
    nc.vector.reciprocal(out=rstd, in_=rstd)
    nc.vector.tensor_scalar(
        out=out_ap, in0=x_ap, scalar1=mv[:, 0:1], scalar2=rstd,
        op0=mybir.AluOpType.subtract, op1=mybir.AluOpType.mult,
    )


def build_bass():
    nc = bacc.Bacc("TRN2", debug=False)

    x_d = nc.dram_tensor("x", [N_TOK, DIM], F32, kind="ExternalInput")
    qkv_wt_d = nc.dram_tensor("qkv_wt", [DIM, 3 * DIM], BF16, kind="ExternalInput")
    qkb_pm_d = nc.dram_tensor("qkb_pm", [128, MC_QK], F32, kind="ExternalInput")
    vb_d = nc.dram_tensor("vb", [DIM], F32, kind="ExternalInput")
    proj_wt_d = nc.dram_tensor("proj_wt", [DIM, DIM], BF16, kind="ExternalInput")
    projb_d = nc.dram_tensor("projb", [DIM], F32, kind="ExternalInput")
    fc1_wt_d = nc.dram_tensor("fc1_wt", [DIM, HIDDEN], BF16, kind="ExternalInput")
    fc1b_pm_d = nc.dram_tensor("fc1b_pm", [128, MC_H], F32, kind="ExternalInput")
    fc2_wt_d = nc.dram_tensor("fc2_wt", [HIDDEN, DIM], BF16, kind="ExternalInput")
    fc2b_d = nc.dram_tensor("fc2b", [DIM], F32, kind="ExternalInput")
    out_d = nc.dram_tensor("out", [N_TOK, DIM], F32, kind="ExternalOutput")

    x_dt = x_d.ap().rearrange("(t p) c -> p t c", p=128)
    out_dt = out_d.ap().rearrange("(t p) c -> p t c", p=128)
    # weight chunk views: [128 part of in-feat, in-chunk, out-col]
    qkv_w3 = qkv_wt_d.ap().rearrange("(ko p) n -> p ko n", p=128)
    proj_w3 = proj_wt_d.ap().rearrange("(ko p) n -> p ko n", p=128)
    fc1_w3 = fc1_wt_d.ap().rearrange("(ko p) n -> p ko n", p=128)
    fc2_w3 = fc2_wt_d.ap().rearrange("(ko p) n -> p ko n", p=128)

    def bcast128(ap_1d, n):
        return bass.AP(tensor=ap_1d.tensor, offset=ap_1d.offset,
                       ap=[[0, 128], [1, n]])

    with tile.TileContext(nc) as tc:
        with (
            tc.tile_pool(name="const", bufs=1) as const_pool,
            tc.tile_pool(name="resid", bufs=1) as resid_pool,
            tc.tile_pool(name="stats", bufs=4) as stat_pool,
            # PSUM: big 2-bank tiles (S pairs, fc1, batched transposes),
            # small 1-bank tiles (qkv/v/proj/fc2), ctx tiles. 4+2+2 = 8 banks.
            tc.tile_pool(name="psum_big", bufs=2, space="PSUM") as psum_big,
            tc.tile_pool(name="psum_small", bufs=2, space="PSUM") as psum_small,
            tc.tile_pool(name="psum_ctx", bufs=2, space="PSUM") as psum_ctx,
            tc.tile_pool(name="h2fm", bufs=1) as h2fm_pool,
        ):
            x_sb = resid_pool.tile([128, TC, DIM], F32)
            for t in range(TC):
                nc.sync.dma_start(out=x_sb[:, t, :], in_=x_dt[:, t, :])
            ident = const_pool.tile([128, 128], BF16)
            make_identity(nc, ident)
            eps_tile = const_pool.tile([128, 1], F32)
            nc.vector.memset(eps_tile, EPS)
            qkb_pm = const_pool.tile([128, MC_QK], F32)
            nc.sync.dma_start(out=qkb_pm, in_=qkb_pm_d.ap())
            fc1b_pm = const_pool.tile([128, MC_H], F32)
            nc.sync.dma_start(out=fc1b_pm, in_=fc1b_pm_d.ap())
            vb_bc = const_pool.tile([128, DIM], F32)
            nc.sync.dma_start(out=vb_bc, in_=bcast128(vb_d.ap(), DIM))
            projb_bc = const_pool.tile([128, DIM], F32)
            nc.sync.dma_start(out=projb_bc, in_=bcast128(projb_d.ap(), DIM))
            fc2b_bc = const_pool.tile([128, DIM], F32)
            nc.sync.dma_start(out=fc2b_bc, in_=bcast128(fc2b_d.ap(), DIM))
            ones_bf = const_pool.tile([128, 96], BF16)
            nc.vector.memset(ones_bf, 1.0)

            h2_fm = h2fm_pool.tile([128, FC, N_TOK], BF16, tag="hfm2")

            def ln_transpose(t, dst_fm):
                """LN of token chunk t + PE-transpose into dst_fm[:, :, t*128:]."""
                h_tile = stat_pool.tile([128, DIM], BF16, tag="h_tile")
                _ln_chunk(nc, stat_pool, eps_tile, x_sb[:, t, :], h_tile)
                tr = psum_big.tile([128, FC, 128], BF16, tag="big")
                for f in range(FC):
                    nc.tensor.transpose(
                        tr[:, f, :], h_tile[:, f * 128:(f + 1) * 128], ident)
                nc.scalar.copy(
                    out=dst_fm[:, :, t * 128:(t + 1) * 128], in_=tr)

            # ============ attention region (qkv + attention + proj) =========
            with (
                tc.tile_pool(name="qk", bufs=1) as qk_pool,
                tc.tile_pool(name="vaug", bufs=1) as v_pool,
                tc.tile_pool(name="ctxfm", bufs=1) as ctx_pool,
                tc.tile_pool(name="wproj", bufs=2) as wproj_pool,
            ):
                qk_fm = qk_pool.tile([128, MC_QK, N_TOK], BF16)
                v_aug = v_pool.tile([128, TC, HEADS, HD + 1], BF16)
                ctx_fm = ctx_pool.tile([128, FC, N_TOK], BF16)

                with (
                    tc.tile_pool(name="hfm", bufs=1) as hfm_pool,
                    tc.tile_pool(name="wqkv", bufs=3) as wqkv_pool,
                    tc.tile_pool(name="wvp", bufs=2) as wv_pool,
                    tc.tile_pool(name="exps", bufs=25) as exps_pool,
                    tc.tile_pool(name="dsmall", bufs=2) as dsmall_pool,
                ):
                    h_fm = hfm_pool.tile([128, FC, N_TOK], BF16, tag="hfm")

                    # v weights resident: [128, 6, 384] x2
                    wv = [wv_pool.tile([128, FC, 384], BF16, tag="wv",
                                       name=f"wv{i}") for i in range(2)]
                    for nv in range(2):
                        nc.sync.dma_start(
                            out=wv[nv],
                            in_=qkv_w3[:, :, 2 * DIM + nv * 384:
                                       2 * DIM + (nv + 1) * 384])
                    nc.scalar.copy(
                        out=v_aug[:, :, :, HD],
                        in_=ones_bf[:, 0:96].rearrange("p (a b) -> p a b", a=TC))

                    # LN1 + v per token chunk (v starts the PE early)
                    for t in range(TC):
                        ln_transpose(t, h_fm)
                        for nv in range(2):
                            ps = psum_small.tile([128, 384], F32, tag="sm")
                            for k in range(FC):
                                nc.tensor.matmul(
                                    ps, h_fm[:, k, t * 128:(t + 1) * 128],
                                    wv[nv][:, k, :],
                                    start=(k == 0), stop=(k == FC - 1))
                            nc.vector.tensor_add(
                                out=v_aug[:, t, nv * 6:(nv + 1) * 6, 0:HD],
                                in0=ps.rearrange("p (h d) -> p h d", d=HD),
                                in1=vb_bc[:, nv * 384:(nv + 1) * 384].rearrange(
                                    "p (h d) -> p h d", d=HD))

                    def emit_qk(m):
                        wt = wqkv_pool.tile([128, FC, 128], BF16, tag="wqkv")
                        nc.sync.dma_start(
                            out=wt, in_=qkv_w3[:, :, m * 128:(m + 1) * 128])
                        for q in range(2):
                            ps = psum_small.tile([128, 512], F32, tag="sm")
                            for k in range(FC):
                                nc.tensor.matmul(
                                    ps, wt[:, k, :],
                                    h_fm[:, k, q * 512:(q + 1) * 512],
                                    start=(k == 0), stop=(k == FC - 1))
                            nc.vector.tensor_scalar_add(
                                out=qk_fm[:, m, q * 512:(q + 1) * 512], in0=ps,
                                scalar1=qkb_pm[:, m:m + 1])

                    def emit_s_exp(j):
                        """S + exp for head pair (2j, 2j+1); returns exps."""
                        exps = []  # [kc][ab]
                        for kc in range(TC):
                            sps = []
                            for ab in range(2):
                                po = 64 * ab
                                sp = psum_big.tile([128, N_TOK], F32, tag="big")
                                for q in range(2):
                                    nc.tensor.matmul(
                                        sp[:, q * 512:(q + 1) * 512],
                                        qk_fm[po:po + 64, 6 + j,
                                              kc * 128:(kc + 1) * 128],
                                        qk_fm[po:po + 64, j,
                                              q * 512:(q + 1) * 512],
                                        start=True, stop=True)
                                sps.append(sp)
                            pair = []
                            for ab in range(2):
                                e_t = exps_pool.tile([128, N_TOK], BF16,
                                                     tag="exps")
                                nc.scalar.activation(
                                    out=e_t, in_=sps[ab],
                                    func=mybir.ActivationFunctionType.Exp,
                                    scale=SCALE)
                                pair.append(e_t)
                            exps.append(pair)
                        return exps

                    def emit_ctx(j, exps):
                        for ab in range(2):
                            h = 2 * j + ab
                            po = 64 * ab
                            for q in range(2):
                                cp = psum_ctx.tile([65, 512], F32, tag="ctx")
                                for kc in range(TC):
                                    nc.tensor.matmul(
                                        cp, v_aug[:, kc, h, :],
                                        exps[kc][ab][:, q * 512:(q + 1) * 512],
                                        start=(kc == 0), stop=(kc == TC - 1))
                                cu = dsmall_pool.tile([128, 512], F32,
                                                      tag="cu")
                                nc.scalar.copy(out=cu[po:po + 64, :],
                                               in_=cp[0:64, :])
                                den = dsmall_pool.tile([1, 512], F32, tag="den")
                                nc.scalar.copy(out=den, in_=cp[64:65, :])
                                scr = dsmall_pool.tile([1, 512], F32, tag="scr")
                                rec = dsmall_pool.tile([1, 512], F32, tag="rec")
                                nc.vector.reciprocal_approx_accurate(
                                    out=rec, in_=den, scratch=scr)
                                bcd = dsmall_pool.tile([128, 512], F32,
                                                       tag="bcd")
                                nc.gpsimd.partition_broadcast(bcd, rec)
                                nc.vector.tensor_tensor(
                                    out=ctx_fm[po:po + 64, h // 2,
                                               q * 512:(q + 1) * 512],
                                    in0=cu[po:po + 64, :],
                                    in1=bcd[po:po + 64, :],
                                    op=mybir.AluOpType.mult)

                    pend = None
                    for j in range(6):
                        emit_qk(j)
                        emit_qk(6 + j)
                        exps = emit_s_exp(j)
                        if pend is not None:
                            emit_ctx(*pend)
                        pend = (j, exps)
                        if j == 4:
                            wp = [wproj_pool.tile([128, FC, 384], BF16,
                                                  tag="wp", name=f"wp{i}")
                                  for i in range(2)]
                            for nv in range(2):
                                nc.sync.dma_start(
                                    out=wp[nv],
                                    in_=proj_w3[:, :, nv * 384:(nv + 1) * 384])
                    emit_ctx(*pend)

                # ---------------- proj + residual + LN2 ----------------
                if True:
                    for t in range(TC):
                        for nv in range(2):
                            ps = psum_small.tile([128, 384], F32, tag="sm")
                            for k in range(FC):
                                nc.tensor.matmul(
                                    ps, ctx_fm[:, k, t * 128:(t + 1) * 128],
                                    wp[nv][:, k, :],
                                    start=(k == 0), stop=(k == FC - 1))
                            sl = slice(nv * 384, (nv + 1) * 384)
                            nc.vector.tensor_add(
                                out=x_sb[:, t, sl], in0=ps, in1=x_sb[:, t, sl])
                            nc.vector.tensor_add(
                                out=x_sb[:, t, sl], in0=x_sb[:, t, sl],
                                in1=projb_bc[:, sl])
                        ln_transpose(t, h2_fm)

            # ---------------- MLP: fc1 / gelu / fc2 ----------------
            with (
                tc.tile_pool(name="gfm", bufs=1) as g_pool,
                tc.tile_pool(name="wfc1", bufs=1) as wfc1_pool,
                tc.tile_pool(name="wfc2", bufs=2) as wfc2_pool,
                tc.tile_pool(name="outt", bufs=4) as out_pool,
            ):
                g_fm = g_pool.tile([128, MC_H, N_TOK], BF16)
                wf1 = wfc1_pool.tile([128, FC, HIDDEN], BF16)
                nc.sync.dma_start(out=wf1, in_=fc1_w3)
                wf2 = [wfc2_pool.tile([128, MC_H, 384], BF16,
                                      tag="wf2", name=f"wf2{i}")
                       for i in range(2)]
                for nv in range(2):
                    nc.sync.dma_start(
                        out=wf2[nv],
                        in_=fc2_w3[:, :, nv * 384:(nv + 1) * 384])

                for m in range(MC_H):
                    ps = psum_big.tile([128, N_TOK], F32, tag="big")
                    for half in range(2):
                        for k in range(FC):
                            nc.tensor.matmul(
                                ps[:, half * 512:(half + 1) * 512],
                                wf1[:, k, m * 128:(m + 1) * 128],
                                h2_fm[:, k, half * 512:(half + 1) * 512],
                                start=(k == 0), stop=(k == FC - 1))
                    nc.scalar.activation(
                        out=g_fm[:, m, :], in_=ps,
                        func=mybir.ActivationFunctionType.Gelu,
                        bias=fc1b_pm[:, m:m + 1], scale=1.0)

                for t in range(TC):
                    for nv in range(2):
                        ps = psum_small.tile([128, 384], F32, tag="sm")
                        for k in range(MC_H):
                            nc.tensor.matmul(
                                ps, g_fm[:, k, t * 128:(t + 1) * 128],
                                wf2[nv][:, k, :],
                                start=(k == 0), stop=(k == MC_H - 1))
                        sl = slice(nv * 384, (nv + 1) * 384)
                        o_t = out_pool.tile([128, 384], F32, tag="outt")
                        nc.vector.tensor_add(out=o_t, in0=ps, in1=x_sb[:, t, sl])
                        nc.vector.tensor_add(out=o_t, in0=o_t, in1=fc2b_bc[:, sl])
                        nc.sync.dma_start(out=out_dt[:, t, sl], in_=o_t)

    nc.compile()
    return nc


def host_prep(x, ln1_g, ln1_b, qkv_w, proj_w, proj_b, ln2_g, ln2_b,
              fc1_w, fc1_b, fc2_w, fc2_b):
    """Fold LN affine params into weights, pre-transpose, cast to bf16."""
    import ml_dtypes
    f32 = np.float32
    bf16 = ml_dtypes.bfloat16
    qkv_w = np.asarray(qkv_w, f32)
    qkv_wt = np.ascontiguousarray(
        (qkv_w * np.asarray(ln1_g, f32)[None, :]).T).astype(bf16)
    qkv_bias = qkv_w @ np.asarray(ln1_b, f32)
    qkb_pm = np.ascontiguousarray(qkv_bias[:2 * DIM].reshape(MC_QK, 128).T)
    vb = np.ascontiguousarray(qkv_bias[2 * DIM:])
    proj_wt = np.ascontiguousarray(np.asarray(proj_w, f32).T).astype(bf16)
    fc1_w = np.asarray(fc1_w, f32)
    fc1_wt = np.ascontiguousarray(
        (fc1_w * np.asarray(ln2_g, f32)[None, :]).T).astype(bf16)
    fc1_bias = fc1_w @ np.asarray(ln2_b, f32) + np.asarray(fc1_b, f32)
    fc1b_pm = np.ascontiguousarray(fc1_bias.reshape(MC_H, 128).T)
    fc2_wt = np.ascontiguousarray(np.asarray(fc2_w, f32).T).astype(bf16)
    return {
        "qkv_wt": qkv_wt, "qkb_pm": qkb_pm, "vb": vb,
        "proj_wt": proj_wt, "projb": np.ascontiguousarray(np.asarray(proj_b, f32)),
        "fc1_wt": fc1_wt, "fc1b_pm": fc1b_pm,
        "fc2_wt": fc2_wt, "fc2b": np.ascontiguousarray(np.asarray(fc2_b, f32)),
    }


_CACHE = {}


def kernel(x, ln1_g, ln1_b, qkv_w, proj_w, proj_b, ln2_g, ln2_b,
           fc1_w, fc1_b, fc2_w, fc2_b, _want_results=False, **_ignored):
    from concourse.bass_utils import run_bass_kernel_spmd

    x = np.asarray(x, np.float32)
    B = x.shape[0]
    assert B == 8 and x.shape[1] == N_TOK and x.shape[2] == DIM

    w = host_prep(x, ln1_g, ln1_b, qkv_w, proj_w, proj_b, ln2_g, ln2_b,
                  fc1_w, fc1_b, fc2_w, fc2_b)

    if "nc" not in _CACHE:
        _CACHE["nc"] = build_bass()
    nc = _CACHE["nc"]

    in_maps = [dict(w, x=np.ascontiguousarray(x[i])) for i in range(B)]
    res = run_bass_kernel_spmd(nc, in_maps, core_ids=list(range(B)))
    out = np.stack([res.results[i]["out"] for i in range(B)], axis=0)
    if _want_results:
        return out, res
    return out


# revision 29
# speedup vs baseline: 1.6933x; 1.0147x over previous
"""Trainium2 Bass kernel for a dense transformer block (pre-LN, 12 heads, MLP 4x).

Strategy: data-parallel over batch across the 8 NeuronCores (B=8 -> one batch
element per core, no collectives). Per core:

  - residual stream kept token-major fp32 [128 tok x 768] (8 token chunks)
  - LN on DVE via bn_stats/bn_aggr; LN affine params folded into the weights
    on the host; matmul operands bf16, PSUM accumulation fp32
  - h transposed to feature-major via PE transposes batched 6-to-a-PSUM-bank
  - QKV: q,k produced feature-major [64d x tok per head]; v produced
    token-major directly (no on-chip transposes for v); one 3D-AP DMA per
    weight chunk group
  - attention computed transposed: S_t[k,q] = k_fm.T @ q_fm with the two heads
    of a pair packed into disjoint PE row groups (base partitions 0/64) so
    their matmuls run concurrently; exp on ACT over [128,1024] PSUM tiles with
    fused 1/sqrt(d); softmax denominator from a ones-row appended to V; the
    normalization is gpsimd partition_broadcast + one DVE mult straight out
    of PSUM. S/exp for a pair is emitted inside the QKV loop right after its
    q/k chunks, and ctx of the previous pair follows, so PE and ACT overlap
    across the whole attention region.
  - MLP: fc1 -> ACT gelu (exact, fused bias) -> fc2, with LN2 interleaved
    into the proj loop
"""

import numpy as np

import concourse.bass as bass
import concourse.mybir as mybir
import concourse.tile as tile
from concourse import bacc
from concourse.masks import make_identity

DIM = 768
HEADS = 12
HD = 64  # head dim
HIDDEN = 3072
N_TOK = 1024
TC = N_TOK // 128  # 8 token chunks
FC = DIM // 128  # 6 feature chunks
MC_QK = 2 * DIM // 128  # 12 chunks of q|k features
MC_H = HIDDEN // 128  # 24 hidden chunks
EPS = 1e-5
SCALE = HD ** -0.5

F32 = mybir.dt.float32
BF16 = mybir.dt.bfloat16


def _ln_chunk(nc, stat_pool, eps_tile, x_ap, out_ap):
    """out = (x - mean(x)) * rsqrt(var(x) + eps), row-wise over 768."""
    stats = stat_pool.tile([128, 3, 6], F32, tag="ln_stats")
    for sg in range(3):
        nc.vector.bn_stats(out=stats[:, sg, :], in_=x_ap[:, sg * 256:(sg + 1) * 256])
    mv = stat_pool.tile([128, 2], F32, tag="ln_mv")
    nc.vector.bn_aggr(out=mv, in_=stats)
    rstd = stat_pool.tile([128, 1], F32, tag="ln_rstd")
    nc.scalar.activation(
        out=rstd, in_=mv[:, 1:2], func=mybir.ActivationFunctionType.Sqrt,
        bias=eps_tile, scale=1.0,
    )
    nc.vector.reciprocal(out=rstd, in_=rstd)
    nc.vector.tensor_scalar(
        out=out_ap, in0=x_ap, scalar1=mv[:, 0:1], scalar2=rstd,
        op0=mybir.AluOpType.subtract, op1=mybir.AluOpType.mult,
    )


def build_bass():
    nc = bacc.Bacc("TRN2", debug=False)

    x_d = nc.dram_tensor("x", [N_TOK, DIM], F32, kind="ExternalInput")
    qkv_wt_d = nc.dram_tensor("qkv_wt", [DIM, 3 * DIM], BF16, kind="ExternalInput")
    qkb_pm_d = nc.dram_tensor("qkb_pm", [128, MC_QK], F32, kind="ExternalInput")
    vb_d = nc.dram_tensor("vb", [DIM], F32, kind="ExternalInput")
    proj_wt_d = nc.dram_tensor("proj_wt", [DIM, DIM], BF16, kind="ExternalInput")
    projb_d = nc.dram_tensor("projb", [DIM], F32, kind="ExternalInput")
    fc1_wt_d = nc.dram_tensor("fc1_wt", [DIM, HIDDEN], BF16, kind="ExternalInput")
    fc1b_pm_d = nc.dram_tensor("fc1b_pm", [128, MC_H], F32, kind="ExternalInput")
    fc2_wt_d = nc.dram_tensor("fc2_wt", [HIDDEN, DIM], BF16, kind="ExternalInput")
    fc2b_d = nc.dram_tensor("fc2b", [DIM], F32, kind="ExternalInput")
    out_d = nc.dram_tensor("out", [N_TOK, DIM], F32, kind="ExternalOutput")

    x_dt = x_d.ap().rearrange("(t p) c -> p t c", p=128)
    out_dt = out_d.ap().rearrange("(t p) c -> p t c", p=128)
    # weight chunk views: [128 part of in-feat, in-chunk, out-col]
    qkv_w3 = qkv_wt_d.ap().rearrange("(ko p) n -> p ko n", p=128)
    proj_w3 = proj_wt_d.ap().rearrange("(ko p) n -> p ko n", p=128)
    fc1_w3 = fc1_wt_d.ap().rearrange("(ko p) n -> p ko n", p=128)
    fc2_w3 = fc2_wt_d.ap().rearrange("(ko p) n -> p ko n", p=128)

    def bcast128(ap_1d, n):
        return bass.AP(tensor=ap_1d.tensor, offset=ap_1d.offset,
                       ap=[[0, 128], [1, n]])

    with tile.TileContext(nc) as tc:
        with (
            tc.tile_pool(name="const", bufs=1) as const_pool,
            tc.tile_pool(name="resid", bufs=1) as resid_pool,
            tc.tile_pool(name="stats", bufs=4) as stat_pool,
            # PSUM: big 2-bank tiles (S pairs, fc1, batched transposes),
            # small 1-bank tiles (qkv/v/proj/fc2), ctx tiles. 4+2+2 = 8 banks.
            tc.tile_pool(name="psum_big", bufs=2, space="PSUM") as psum_big,
            tc.tile_pool(name="psum_small", bufs=2, space="PSUM") as psum_small,
            tc.tile_pool(name="psum_ctx", bufs=2, space="PSUM") as psum_ctx,
            tc.tile_pool(name="h2fm", bufs=1) as h2fm_pool,
        ):
            x_sb = resid_pool.tile([128, TC, DIM], F32)
            for t in range(TC):
                nc.sync.dma_start(out=x_sb[:, t, :], in_=x_dt[:, t, :])
            ident = const_pool.tile([128, 128], BF16)
            make_identity(nc, ident)
            eps_tile = const_pool.tile([128, 1], F32)
            nc.vector.memset(eps_tile, EPS)
            qkb_pm = const_pool.tile([128, MC_QK], F32)
            nc.sync.dma_start(out=qkb_pm, in_=qkb_pm_d.ap())
            fc1b_pm = const_pool.tile([128, MC_H], F32)
            nc.sync.dma_start(out=fc1b_pm, in_=fc1b_pm_d.ap())
            vb_bc = const_pool.tile([128, DIM], F32)
            nc.sync.dma_start(out=vb_bc, in_=bcast128(vb_d.ap(), DIM))
            projb_bc = const_pool.tile([128, DIM], F32)
            nc.sync.dma_start(out=projb_bc, in_=bcast128(projb_d.ap(), DIM))
            fc2b_bc = const_pool.tile([128, DIM], F32)
            nc.sync.dma_start(out=fc2b_bc, in_=bcast128(fc2b_d.ap(), DIM))
            ones_bf = const_pool.tile([128, 128], BF16)
            nc.vector.memset(ones_bf, 1.0)
            projb_row = const_pool.tile([1, DIM], BF16)
            nc.vector.tensor_copy(out=projb_row, in_=projb_bc[0:1, :])
            fc2b_row = const_pool.tile([1, DIM], BF16)
            nc.vector.tensor_copy(out=fc2b_row, in_=fc2b_bc[0:1, :])

            h2_fm = h2fm_pool.tile([128, FC, N_TOK], BF16, tag="hfm2")

            def ln_transpose(t, dst_fm):
                """LN of token chunk t + PE-transpose into dst_fm[:, :, t*128:]."""
                h_tile = stat_pool.tile([128, DIM], BF16, tag="h_tile")
                _ln_chunk(nc, stat_pool, eps_tile, x_sb[:, t, :], h_tile)
                tr = psum_big.tile([128, FC, 128], BF16, tag="big")
                for f in range(FC):
                    nc.tensor.transpose(
                        tr[:, f, :], h_tile[:, f * 128:(f + 1) * 128], ident)
                nc.scalar.copy(
                    out=dst_fm[:, :, t * 128:(t + 1) * 128], in_=tr)

            # ============ attention region (qkv + attention + proj) =========
            with (
                tc.tile_pool(name="qk", bufs=1) as qk_pool,
                tc.tile_pool(name="vaug", bufs=1) as v_pool,
                tc.tile_pool(name="ctxfm", bufs=1) as ctx_pool,
                tc.tile_pool(name="wproj", bufs=2) as wproj_pool,
            ):
                qk_fm = qk_pool.tile([128, MC_QK, N_TOK], BF16)
                v_aug = v_pool.tile([128, TC, HEADS, HD + 1], BF16)
                ctx_fm = ctx_pool.tile([128, FC, N_TOK], BF16)

                with (
                    tc.tile_pool(name="hfm", bufs=1) as hfm_pool,
                    tc.tile_pool(name="wqkv", bufs=3) as wqkv_pool,
                    tc.tile_pool(name="wvp", bufs=2) as wv_pool,
                    tc.tile_pool(name="exps", bufs=24) as exps_pool,
                    tc.tile_pool(name="dsmall", bufs=2) as dsmall_pool,
                ):
                    h_fm = hfm_pool.tile([128, FC, N_TOK], BF16, tag="hfm")

                    # v weights resident: [128, 6, 384] x2
                    wv = [wv_pool.tile([128, FC, 384], BF16, tag="wv",
                                       name=f"wv{i}") for i in range(2)]
                    for nv in range(2):
                        nc.sync.dma_start(
                            out=wv[nv],
                            in_=qkv_w3[:, :, 2 * DIM + nv * 384:
                                       2 * DIM + (nv + 1) * 384])
                    nc.scalar.copy(
                        out=v_aug[:, :, :, HD],
                        in_=ones_bf[:, 0:96].rearrange("p (a b) -> p a b", a=TC))

                    # LN1 + v per token chunk (v starts the PE early)
                    for t in range(TC):
                        ln_transpose(t, h_fm)
                        for nv in range(2):
                            ps = psum_small.tile([128, 384], F32, tag="sm")
                            for k in range(FC):
                                nc.tensor.matmul(
                                    ps, h_fm[:, k, t * 128:(t + 1) * 128],
                                    wv[nv][:, k, :],
                                    start=(k == 0), stop=(k == FC - 1))
                            nc.vector.tensor_add(
                                out=v_aug[:, t, nv * 6:(nv + 1) * 6, 0:HD],
                                in0=ps.rearrange("p (h d) -> p h d", d=HD),
                                in1=vb_bc[:, nv * 384:(nv + 1) * 384].rearrange(
                                    "p (h d) -> p h d", d=HD))

                    def emit_qk(m):
                        wt = wqkv_pool.tile([128, FC, 128], BF16, tag="wqkv")
                        nc.sync.dma_start(
                            out=wt, in_=qkv_w3[:, :, m * 128:(m + 1) * 128])
                        for q in range(2):
                            ps = psum_small.tile([128, 512], F32, tag="sm")
                            for k in range(FC):
                                nc.tensor.matmul(
                                    ps, wt[:, k, :],
                                    h_fm[:, k, q * 512:(q + 1) * 512],
                                    start=(k == 0), stop=(k == FC - 1))
                            nc.vector.tensor_scalar_add(
                                out=qk_fm[:, m, q * 512:(q + 1) * 512], in0=ps,
                                scalar1=qkb_pm[:, m:m + 1])

                    def emit_s_exp(j):
                        """S + exp for head pair (2j, 2j+1); returns exps."""
                        exps = []  # [kc][ab]
                        for kc in range(TC):
                            sps = []
                            for ab in range(2):
                                po = 64 * ab
                                sp = psum_big.tile([128, N_TOK], F32, tag="big")
                                for q in range(2):
                                    nc.tensor.matmul(
                                        sp[:, q * 512:(q + 1) * 512],
                                        qk_fm[po:po + 64, 6 + j,
                                              kc * 128:(kc + 1) * 128],
                                        qk_fm[po:po + 64, j,
                                              q * 512:(q + 1) * 512],
                                        start=True, stop=True)
                                sps.append(sp)
                            pair = []
                            for ab in range(2):
                                e_t = exps_pool.tile([128, N_TOK], BF16,
                                                     tag="exps")
                                nc.scalar.activation(
                                    out=e_t, in_=sps[ab],
                                    func=mybir.ActivationFunctionType.Exp,
                                    scale=SCALE)
                                pair.append(e_t)
                            exps.append(pair)
                        return exps

                    def emit_ctx(j, exps):
                        for ab in range(2):
                            h = 2 * j + ab
                            po = 64 * ab
                            for q in range(2):
                                cp = psum_ctx.tile([65, 512], F32, tag="ctx")
                                for kc in range(TC):
                                    nc.tensor.matmul(
                                        cp, v_aug[:, kc, h, :],
                                        exps[kc][ab][:, q * 512:(q + 1) * 512],
                                        start=(kc == 0), stop=(kc == TC - 1))
                                cu = dsmall_pool.tile([128, 512], F32,
                                                      tag="cu")
                                nc.vector.tensor_copy(out=cu[po:po + 64, :],
                                                      in_=cp[0:64, :])
                                den = dsmall_pool.tile([1, 512], F32, tag="den")
                                nc.scalar.copy(out=den, in_=cp[64:65, :])
                                scr = dsmall_pool.tile([1, 512], F32, tag="scr")
                                rec = dsmall_pool.tile([1, 512], F32, tag="rec")
                                nc.vector.reciprocal_approx_accurate(
                                    out=rec, in_=den, scratch=scr)
                                bcd = dsmall_pool.tile([128, 512], F32,
                                                       tag="bcd")
                                nc.gpsimd.partition_broadcast(bcd, rec)
                                nc.vector.tensor_tensor(
                                    out=ctx_fm[po:po + 64, h // 2,
                                               q * 512:(q + 1) * 512],
                                    in0=cu[po:po + 64, :],
                                    in1=bcd[po:po + 64, :],
                                    op=mybir.AluOpType.mult)

                    pend = None
                    for j in range(6):
                        emit_qk(j)
                        emit_qk(6 + j)
                        exps = emit_s_exp(j)
                        if pend is not None:
                            emit_ctx(*pend)
                        pend = (j, exps)
                        if j == 4:
                            wp = [wproj_pool.tile([128, FC, 384], BF16,
                                                  tag="wp", name=f"wp{i}")
                                  for i in range(2)]
                            for nv in range(2):
                                nc.sync.dma_start(
                                    out=wp[nv],
                                    in_=proj_w3[:, :, nv * 384:(nv + 1) * 384])
                    emit_ctx(*pend)

                # ---------------- proj + residual + LN2 ----------------
                if True:
                    for t in range(TC):
                        for nv in range(2):
                            ps = psum_small.tile([128, 384], F32, tag="sm")
                            for k in range(FC):
                                nc.tensor.matmul(
                                    ps, ctx_fm[:, k, t * 128:(t + 1) * 128],
                                    wp[nv][:, k, :],
                                    start=(k == 0), stop=False)
                            sl = slice(nv * 384, (nv + 1) * 384)
                            nc.tensor.matmul(
                                ps, ones_bf[0:1, :], projb_row[0:1, sl],
                                start=False, stop=True)
                            nc.vector.tensor_add(
                                out=x_sb[:, t, sl], in0=ps, in1=x_sb[:, t, sl])
                        ln_transpose(t, h2_fm)

            # ---------------- MLP: fc1 / gelu / fc2 ----------------
            with (
                tc.tile_pool(name="gfm", bufs=1) as g_pool,
                tc.tile_pool(name="wfc1", bufs=1) as wfc1_pool,
                tc.tile_pool(name="wfc2", bufs=2) as wfc2_pool,
                tc.tile_pool(name="outt", bufs=4) as out_pool,
            ):
                g_fm = g_pool.tile([128, MC_H, N_TOK], BF16)
                wf1 = wfc1_pool.tile([128, FC, HIDDEN], BF16)
                nc.sync.dma_start(out=wf1, in_=fc1_w3)
                wf2 = [wfc2_pool.tile([128, MC_H, 384], BF16,
                                      tag="wf2", name=f"wf2{i}")
                       for i in range(2)]
                for nv in range(2):
                    nc.sync.dma_start(
                        out=wf2[nv],
                        in_=fc2_w3[:, :, nv * 384:(nv + 1) * 384])

                for m in range(MC_H):
                    ps = psum_big.tile([128, N_TOK], F32, tag="big")
                    for half in range(2):
                        for k in range(FC):
                            nc.tensor.matmul(
                                ps[:, half * 512:(half + 1) * 512],
                                wf1[:, k, m * 128:(m + 1) * 128],
                                h2_fm[:, k, half * 512:(half + 1) * 512],
                                start=(k == 0), stop=(k == FC - 1))
                    nc.scalar.activation(
                        out=g_fm[:, m, :], in_=ps,
                        func=mybir.ActivationFunctionType.Gelu,
                        bias=fc1b_pm[:, m:m + 1], scale=1.0)

                for t in range(TC):
                    for nv in range(2):
                        ps = psum_small.tile([128, 384], F32, tag="sm")
                        for k in range(MC_H):
                            nc.tensor.matmul(
                                ps, g_fm[:, k, t * 128:(t + 1) * 128],
                                wf2[nv][:, k, :],
                                start=(k == 0), stop=False)
                        sl = slice(nv * 384, (nv + 1) * 384)
                        nc.tensor.matmul(
                            ps, ones_bf[0:1, :], fc2b_row[0:1, sl],
                            start=False, stop=True)
                        o_t = out_pool.tile([128, 384], F32, tag="outt")
                        nc.vector.tensor_add(out=o_t, in0=ps, in1=x_sb[:, t, sl])
                        nc.sync.dma_start(out=out_dt[:, t, sl], in_=o_t)

    nc.compile()
    return nc


def host_prep(x, ln1_g, ln1_b, qkv_w, proj_w, proj_b, ln2_g, ln2_b,
              fc1_w, fc1_b, fc2_w, fc2_b):
    """Fold LN affine params into weights, pre-transpose, cast to bf16."""
    import ml_dtypes
    f32 = np.float32
    bf16 = ml_dtypes.bfloat16
    qkv_w = np.asarray(qkv_w, f32)
    qkv_wt = np.ascontiguousarray(
        (qkv_w * np.asarray(ln1_g, f32)[None, :]).T).astype(bf16)
    qkv_bias = qkv_w @ np.asarray(ln1_b, f32)
    qkb_pm = np.ascontiguousarray(qkv_bias[:2 * DIM].reshape(MC_QK, 128).T)
    vb = np.ascontiguousarray(qkv_bias[2 * DIM:])
    proj_wt = np.ascontiguousarray(np.asarray(proj_w, f32).T).astype(bf16)
    fc1_w = np.asarray(fc1_w, f32)
    fc1_wt = np.ascontiguousarray(
        (fc1_w * np.asarray(ln2_g, f32)[None, :]).T).astype(bf16)
    fc1_bias = fc1_w @ np.asarray(ln2_b, f32) + np.asarray(fc1_b, f32)
    fc1b_pm = np.ascontiguousarray(fc1_bias.reshape(MC_H, 128).T)
    fc2_wt = np.ascontiguousarray(np.asarray(fc2_w, f32).T).astype(bf16)
    return {
        "qkv_wt": qkv_wt, "qkb_pm": qkb_pm, "vb": vb,
        "proj_wt": proj_wt, "projb": np.ascontiguousarray(np.asarray(proj_b, f32)),
        "fc1_wt": fc1_wt, "fc1b_pm": fc1b_pm,
        "fc2_wt": fc2_wt, "fc2b": np.ascontiguousarray(np.asarray(fc2_b, f32)),
    }


_CACHE = {}


def kernel(x, ln1_g, ln1_b, qkv_w, proj_w, proj_b, ln2_g, ln2_b,
           fc1_w, fc1_b, fc2_w, fc2_b, _want_results=False, **_ignored):
    from concourse.bass_utils import run_bass_kernel_spmd

    x = np.asarray(x, np.float32)
    B = x.shape[0]
    assert B == 8 and x.shape[1] == N_TOK and x.shape[2] == DIM

    w = host_prep(x, ln1_g, ln1_b, qkv_w, proj_w, proj_b, ln2_g, ln2_b,
                  fc1_w, fc1_b, fc2_w, fc2_b)

    if "nc" not in _CACHE:
        _CACHE["nc"] = build_bass()
    nc = _CACHE["nc"]

    in_maps = [dict(w, x=np.ascontiguousarray(x[i])) for i in range(B)]
    res = run_bass_kernel_spmd(nc, in_maps, core_ids=list(range(B)))
    out = np.stack([res.results[i]["out"] for i in range(B)], axis=0)
    if _want_results:
        return out, res
    return out


# revision 30
# speedup vs baseline: 1.7622x; 1.0407x over previous
"""Trainium2 Bass kernel for a dense transformer block (pre-LN, 12 heads, MLP 4x).

Strategy: data-parallel over batch across the 8 NeuronCores (B=8 -> one batch
element per core, no collectives). Per core:

  - residual stream kept token-major fp32 [128 tok x 768] (8 token chunks)
  - LN on DVE via bn_stats/bn_aggr; LN affine params folded into the weights
    on the host; matmul operands bf16, PSUM accumulation fp32
  - h transposed to feature-major via PE transposes batched 6-to-a-PSUM-bank
  - QKV: q,k produced feature-major [64d x tok per head]; v produced
    token-major directly (no on-chip transposes for v); one 3D-AP DMA per
    weight chunk group
  - attention computed transposed: S_t[k,q] = k_fm.T @ q_fm with the two heads
    of a pair packed into disjoint PE row groups (base partitions 0/64) so
    their matmuls run concurrently; exp on ACT over [128,1024] PSUM tiles with
    fused 1/sqrt(d); softmax denominator from a ones-row appended to V; the
    normalization is gpsimd partition_broadcast + one DVE mult straight out
    of PSUM. S/exp for a pair is emitted inside the QKV loop right after its
    q/k chunks, and ctx of the previous pair follows, so PE and ACT overlap
    across the whole attention region.
  - MLP: fc1 -> ACT gelu (exact, fused bias) -> fc2, with LN2 interleaved
    into the proj loop
"""

import numpy as np

import concourse.bass as bass
import concourse.mybir as mybir
import concourse.tile as tile
from concourse import bacc
from concourse.masks import make_identity

DIM = 768
HEADS = 12
HD = 64  # head dim
HIDDEN = 3072
N_TOK = 1024
TC = N_TOK // 128  # 8 token chunks
FC = DIM // 128  # 6 feature chunks
MC_QK = 2 * DIM // 128  # 12 chunks of q|k features
MC_H = HIDDEN // 128  # 24 hidden chunks
EPS = 1e-5
SCALE = HD ** -0.5

F32 = mybir.dt.float32
BF16 = mybir.dt.bfloat16


def _ln_chunk(nc, stat_pool, eps_tile, x_ap, out_ap):
    """out = (x - mean(x)) * rsqrt(var(x) + eps), row-wise over 768."""
    stats = stat_pool.tile([128, 3, 6], F32, tag="ln_stats")
    for sg in range(3):
        nc.vector.bn_stats(out=stats[:, sg, :], in_=x_ap[:, sg * 256:(sg + 1) * 256])
    mv = stat_pool.tile([128, 2], F32, tag="ln_mv")
    nc.vector.bn_aggr(out=mv, in_=stats)
    rstd = stat_pool.tile([128, 1], F32, tag="ln_rstd")
    nc.scalar.activation(
        out=rstd, in_=mv[:, 1:2], func=mybir.ActivationFunctionType.Sqrt,
        bias=eps_tile, scale=1.0,
    )
    nc.vector.reciprocal(out=rstd, in_=rstd)
    nc.vector.tensor_scalar(
        out=out_ap, in0=x_ap, scalar1=mv[:, 0:1], scalar2=rstd,
        op0=mybir.AluOpType.subtract, op1=mybir.AluOpType.mult,
    )


def build_bass():
    nc = bacc.Bacc("TRN2", debug=False)

    x_d = nc.dram_tensor("x", [N_TOK, DIM], F32, kind="ExternalInput")
    qkv_wt_d = nc.dram_tensor("qkv_wt", [DIM, 3 * DIM], BF16, kind="ExternalInput")
    qkb_pm_d = nc.dram_tensor("qkb_pm", [128, MC_QK], F32, kind="ExternalInput")
    vb_d = nc.dram_tensor("vb", [DIM], F32, kind="ExternalInput")
    proj_wt_d = nc.dram_tensor("proj_wt", [DIM, DIM], BF16, kind="ExternalInput")
    projb_d = nc.dram_tensor("projb", [DIM], F32, kind="ExternalInput")
    fc1_wt_d = nc.dram_tensor("fc1_wt", [DIM, HIDDEN], BF16, kind="ExternalInput")
    fc1b_pm_d = nc.dram_tensor("fc1b_pm", [128, MC_H], F32, kind="ExternalInput")
    fc2_wt_d = nc.dram_tensor("fc2_wt", [HIDDEN, DIM], BF16, kind="ExternalInput")
    fc2b_d = nc.dram_tensor("fc2b", [DIM], F32, kind="ExternalInput")
    out_d = nc.dram_tensor("out", [N_TOK, DIM], F32, kind="ExternalOutput")

    x_dt = x_d.ap().rearrange("(t p) c -> p t c", p=128)
    out_dt = out_d.ap().rearrange("(t p) c -> p t c", p=128)
    # weight chunk views: [128 part of in-feat, in-chunk, out-col]
    qkv_w3 = qkv_wt_d.ap().rearrange("(ko p) n -> p ko n", p=128)
    proj_w3 = proj_wt_d.ap().rearrange("(ko p) n -> p ko n", p=128)
    fc1_w3 = fc1_wt_d.ap().rearrange("(ko p) n -> p ko n", p=128)
    fc2_w3 = fc2_wt_d.ap().rearrange("(ko p) n -> p ko n", p=128)

    def bcast128(ap_1d, n):
        return bass.AP(tensor=ap_1d.tensor, offset=ap_1d.offset,
                       ap=[[0, 128], [1, n]])

    with tile.TileContext(nc) as tc:
        with (
            tc.tile_pool(name="const", bufs=1) as const_pool,
            tc.tile_pool(name="resid", bufs=1) as resid_pool,
            tc.tile_pool(name="stats", bufs=4) as stat_pool,
            # PSUM: big 2-bank tiles (S pairs, fc1, batched transposes),
            # small 1-bank tiles (qkv/v/proj/fc2), ctx tiles. 4+2+2 = 8 banks.
            tc.tile_pool(name="psum_big", bufs=2, space="PSUM") as psum_big,
            tc.tile_pool(name="psum_small", bufs=2, space="PSUM") as psum_small,
            tc.tile_pool(name="psum_ctx", bufs=2, space="PSUM") as psum_ctx,
            tc.tile_pool(name="h2fm", bufs=1) as h2fm_pool,
        ):
            x_sb = resid_pool.tile([128, TC, DIM], F32)
            for t in range(TC):
                nc.sync.dma_start(out=x_sb[:, t, :], in_=x_dt[:, t, :])
            ident = const_pool.tile([128, 128], BF16)
            make_identity(nc, ident)
            eps_tile = const_pool.tile([128, 1], F32)
            nc.vector.memset(eps_tile, EPS)
            qkb_pm = const_pool.tile([128, MC_QK], F32)
            nc.sync.dma_start(out=qkb_pm, in_=qkb_pm_d.ap())
            fc1b_pm = const_pool.tile([128, MC_H], F32)
            nc.sync.dma_start(out=fc1b_pm, in_=fc1b_pm_d.ap())
            vb_bc = const_pool.tile([128, DIM], F32)
            nc.sync.dma_start(out=vb_bc, in_=bcast128(vb_d.ap(), DIM))
            projb_bc = const_pool.tile([128, DIM], F32)
            nc.sync.dma_start(out=projb_bc, in_=bcast128(projb_d.ap(), DIM))
            fc2b_bc = const_pool.tile([128, DIM], F32)
            nc.sync.dma_start(out=fc2b_bc, in_=bcast128(fc2b_d.ap(), DIM))
            ones_bf = const_pool.tile([128, 128], BF16)
            nc.vector.memset(ones_bf, 1.0)
            projb_row = const_pool.tile([1, DIM], BF16)
            nc.vector.tensor_copy(out=projb_row, in_=projb_bc[0:1, :])
            fc2b_row = const_pool.tile([1, DIM], BF16)
            nc.vector.tensor_copy(out=fc2b_row, in_=fc2b_bc[0:1, :])

            h2_fm = h2fm_pool.tile([128, FC, N_TOK], BF16, tag="hfm2")

            def ln_transpose(t, dst_fm):
                """LN of token chunk t + PE-transpose into dst_fm[:, :, t*128:]."""
                h_tile = stat_pool.tile([128, DIM], BF16, tag="h_tile")
                _ln_chunk(nc, stat_pool, eps_tile, x_sb[:, t, :], h_tile)
                tr = psum_big.tile([128, FC, 128], BF16, tag="big")
                for f in range(FC):
                    nc.tensor.transpose(
                        tr[:, f, :], h_tile[:, f * 128:(f + 1) * 128], ident)
                nc.scalar.copy(
                    out=dst_fm[:, :, t * 128:(t + 1) * 128], in_=tr)

            # ============ attention region (qkv + attention + proj) =========
            with (
                tc.tile_pool(name="qk", bufs=1) as qk_pool,
                tc.tile_pool(name="vaug", bufs=1) as v_pool,
                tc.tile_pool(name="ctxfm", bufs=1) as ctx_pool,
                tc.tile_pool(name="wproj", bufs=2) as wproj_pool,
            ):
                qk_fm = qk_pool.tile([128, MC_QK, N_TOK], BF16)
                v_aug = v_pool.tile([128, TC, HEADS, HD + 1], BF16)
                ctx_fm = ctx_pool.tile([128, FC, N_TOK], BF16)

                with (
                    tc.tile_pool(name="hfm", bufs=1) as hfm_pool,
                    tc.tile_pool(name="wqkv", bufs=3) as wqkv_pool,
                    tc.tile_pool(name="wvp", bufs=2) as wv_pool,
                    tc.tile_pool(name="exps", bufs=24) as exps_pool,
                    tc.tile_pool(name="dsmall", bufs=2) as dsmall_pool,
                ):
                    h_fm = hfm_pool.tile([128, FC, N_TOK], BF16, tag="hfm")

                    # v weights resident: [128, 6, 384] x2
                    wv = [wv_pool.tile([128, FC, 384], BF16, tag="wv",
                                       name=f"wv{i}") for i in range(2)]
                    for nv in range(2):
                        nc.sync.dma_start(
                            out=wv[nv],
                            in_=qkv_w3[:, :, 2 * DIM + nv * 384:
                                       2 * DIM + (nv + 1) * 384])
                    nc.scalar.copy(
                        out=v_aug[:, :, :, HD],
                        in_=ones_bf[:, 0:96].rearrange("p (a b) -> p a b", a=TC))

                    # LN1 + v per token chunk (v starts the PE early)
                    for t in range(TC):
                        ln_transpose(t, h_fm)
                        for nv in range(2):
                            ps = psum_small.tile([128, 384], F32, tag="sm")
                            for k in range(FC):
                                nc.tensor.matmul(
                                    ps, h_fm[:, k, t * 128:(t + 1) * 128],
                                    wv[nv][:, k, :],
                                    start=(k == 0), stop=(k == FC - 1))
                            nc.vector.tensor_add(
                                out=v_aug[:, t, nv * 6:(nv + 1) * 6, 0:HD],
                                in0=ps.rearrange("p (h d) -> p h d", d=HD),
                                in1=vb_bc[:, nv * 384:(nv + 1) * 384].rearrange(
                                    "p (h d) -> p h d", d=HD))

                    def emit_qk(m):
                        wt = wqkv_pool.tile([128, FC, 128], BF16, tag="wqkv")
                        nc.sync.dma_start(
                            out=wt, in_=qkv_w3[:, :, m * 128:(m + 1) * 128])
                        for q in range(2):
                            ps = psum_small.tile([128, 512], F32, tag="sm")
                            for k in range(FC):
                                nc.tensor.matmul(
                                    ps, wt[:, k, :],
                                    h_fm[:, k, q * 512:(q + 1) * 512],
                                    start=(k == 0), stop=(k == FC - 1))
                            nc.vector.tensor_scalar_add(
                                out=qk_fm[:, m, q * 512:(q + 1) * 512], in0=ps,
                                scalar1=qkb_pm[:, m:m + 1])

                    def emit_s_exp(j):
                        """S + exp for head pair (2j, 2j+1); returns exps."""
                        exps = []  # [kc][ab]
                        for kc in range(TC):
                            sps = []
                            for ab in range(2):
                                po = 64 * ab
                                sp = psum_big.tile([128, N_TOK], F32, tag="big")
                                for q in range(2):
                                    nc.tensor.matmul(
                                        sp[:, q * 512:(q + 1) * 512],
                                        qk_fm[po:po + 64, 6 + j,
                                              kc * 128:(kc + 1) * 128],
                                        qk_fm[po:po + 64, j,
                                              q * 512:(q + 1) * 512],
                                        start=True, stop=True)
                                sps.append(sp)
                            pair = []
                            for ab in range(2):
                                e_t = exps_pool.tile([128, N_TOK], BF16,
                                                     tag="exps")
                                nc.scalar.activation(
                                    out=e_t, in_=sps[ab],
                                    func=mybir.ActivationFunctionType.Exp,
                                    scale=SCALE)
                                pair.append(e_t)
                            exps.append(pair)
                        return exps

                    def emit_ctx(j, exps):
                        for ab in range(2):
                            h = 2 * j + ab
                            po = 64 * ab
                            for q in range(2):
                                cp = psum_ctx.tile([65, 512], F32, tag="ctx")
                                for kc in range(TC):
                                    nc.tensor.matmul(
                                        cp, v_aug[:, kc, h, :],
                                        exps[kc][ab][:, q * 512:(q + 1) * 512],
                                        start=(kc == 0), stop=(kc == TC - 1))
                                cu = dsmall_pool.tile([128, 512], F32,
                                                      tag="cu")
                                nc.vector.tensor_copy(out=cu[po:po + 64, :],
                                                      in_=cp[0:64, :])
                                den = dsmall_pool.tile([1, 512], F32, tag="den")
                                nc.vector.tensor_copy(out=den,
                                                      in_=cp[64:65, :])
                                scr = dsmall_pool.tile([1, 512], F32, tag="scr")
                                rec = dsmall_pool.tile([1, 512], F32, tag="rec")
                                nc.vector.reciprocal_approx_accurate(
                                    out=rec, in_=den, scratch=scr)
                                bcd = dsmall_pool.tile([128, 512], F32,
                                                       tag="bcd")
                                nc.gpsimd.partition_broadcast(bcd, rec)
                                nc.vector.tensor_tensor(
                                    out=ctx_fm[po:po + 64, h // 2,
                                               q * 512:(q + 1) * 512],
                                    in0=cu[po:po + 64, :],
                                    in1=bcd[po:po + 64, :],
                                    op=mybir.AluOpType.mult)

                    pend = None
                    for j in range(6):
                        emit_qk(j)
                        emit_qk(6 + j)
                        exps = emit_s_exp(j)
                        if pend is not None:
                            emit_ctx(*pend)
                        pend = (j, exps)
                        if j == 4:
                            wp = [wproj_pool.tile([128, FC, 384], BF16,
                                                  tag="wp", name=f"wp{i}")
                                  for i in range(2)]
                            for nv in range(2):
                                nc.sync.dma_start(
                                    out=wp[nv],
                                    in_=proj_w3[:, :, nv * 384:(nv + 1) * 384])
                    emit_ctx(*pend)

                # ---------------- proj + residual + LN2 ----------------
                if True:
                    for t in range(TC):
                        for nv in range(2):
                            ps = psum_small.tile([128, 384], F32, tag="sm")
                            for k in range(FC):
                                nc.tensor.matmul(
                                    ps, ctx_fm[:, k, t * 128:(t + 1) * 128],
                                    wp[nv][:, k, :],
                                    start=(k == 0), stop=False)
                            sl = slice(nv * 384, (nv + 1) * 384)
                            nc.tensor.matmul(
                                ps, ones_bf[0:1, :], projb_row[0:1, sl],
                                start=False, stop=True)
                            nc.vector.tensor_add(
                                out=x_sb[:, t, sl], in0=ps, in1=x_sb[:, t, sl])
                        ln_transpose(t, h2_fm)

            # ---------------- MLP: fc1 / gelu / fc2 ----------------
            with (
                tc.tile_pool(name="gfm", bufs=1) as g_pool,
                tc.tile_pool(name="wfc1", bufs=6) as wfc1_pool,
                tc.tile_pool(name="wfc2", bufs=2) as wfc2_pool,
                tc.tile_pool(name="outt", bufs=4) as out_pool,
            ):
                g_fm = g_pool.tile([128, MC_H, N_TOK], BF16)
                wf2 = [wfc2_pool.tile([128, MC_H, 384], BF16,
                                      tag="wf2", name=f"wf2{i}")
                       for i in range(2)]
                for nv in range(2):
                    nc.sync.dma_start(
                        out=wf2[nv],
                        in_=fc2_w3[:, :, nv * 384:(nv + 1) * 384])

                for m in range(MC_H):
                    w1t = wfc1_pool.tile([128, FC, 128], BF16, tag="w1t")
                    nc.sync.dma_start(
                        out=w1t, in_=fc1_w3[:, :, m * 128:(m + 1) * 128])
                    ps = psum_big.tile([128, N_TOK], F32, tag="big")
                    for half in range(2):
                        for k in range(FC):
                            nc.tensor.matmul(
                                ps[:, half * 512:(half + 1) * 512],
                                w1t[:, k, :],
                                h2_fm[:, k, half * 512:(half + 1) * 512],
                                start=(k == 0), stop=(k == FC - 1))
                    nc.scalar.activation(
                        out=g_fm[:, m, :], in_=ps,
                        func=mybir.ActivationFunctionType.Gelu,
                        bias=fc1b_pm[:, m:m + 1], scale=1.0)

                for t in range(TC):
                    for nv in range(2):
                        ps = psum_small.tile([128, 384], F32, tag="sm")
                        for k in range(MC_H):
                            nc.tensor.matmul(
                                ps, g_fm[:, k, t * 128:(t + 1) * 128],
                                wf2[nv][:, k, :],
                                start=(k == 0), stop=False)
                        sl = slice(nv * 384, (nv + 1) * 384)
                        nc.tensor.matmul(
                            ps, ones_bf[0:1, :], fc2b_row[0:1, sl],
                            start=False, stop=True)
                        o_t = out_pool.tile([128, 384], F32, tag="outt")
                        nc.vector.tensor_add(out=o_t, in0=ps, in1=x_sb[:, t, sl])
                        nc.sync.dma_start(out=out_dt[:, t, sl], in_=o_t)

    nc.compile()
    return nc


def host_prep(x, ln1_g, ln1_b, qkv_w, proj_w, proj_b, ln2_g, ln2_b,
              fc1_w, fc1_b, fc2_w, fc2_b):
    """Fold LN affine params into weights, pre-transpose, cast to bf16."""
    import ml_dtypes
    f32 = np.float32
    bf16 = ml_dtypes.bfloat16
    qkv_w = np.asarray(qkv_w, f32)
    qkv_wt = np.ascontiguousarray(
        (qkv_w * np.asarray(ln1_g, f32)[None, :]).T).astype(bf16)
    qkv_bias = qkv_w @ np.asarray(ln1_b, f32)
    qkb_pm = np.ascontiguousarray(qkv_bias[:2 * DIM].reshape(MC_QK, 128).T)
    vb = np.ascontiguousarray(qkv_bias[2 * DIM:])
    proj_wt = np.ascontiguousarray(np.asarray(proj_w, f32).T).astype(bf16)
    fc1_w = np.asarray(fc1_w, f32)
    fc1_wt = np.ascontiguousarray(
        (fc1_w * np.asarray(ln2_g, f32)[None, :]).T).astype(bf16)
    fc1_bias = fc1_w @ np.asarray(ln2_b, f32) + np.asarray(fc1_b, f32)
    fc1b_pm = np.ascontiguousarray(fc1_bias.reshape(MC_H, 128).T)
    fc2_wt = np.ascontiguousarray(np.asarray(fc2_w, f32).T).astype(bf16)
    return {
        "qkv_wt": qkv_wt, "qkb_pm": qkb_pm, "vb": vb,
        "proj_wt": proj_wt, "projb": np.ascontiguousarray(np.asarray(proj_b, f32)),
        "fc1_wt": fc1_wt, "fc1b_pm": fc1b_pm,
        "fc2_wt": fc2_wt, "fc2b": np.ascontiguousarray(np.asarray(fc2_b, f32)),
    }


_CACHE = {}


def kernel(x, ln1_g, ln1_b, qkv_w, proj_w, proj_b, ln2_g, ln2_b,
           fc1_w, fc1_b, fc2_w, fc2_b, _want_results=False, **_ignored):
    from concourse.bass_utils import run_bass_kernel_spmd

    x = np.asarray(x, np.float32)
    B = x.shape[0]
    assert B == 8 and x.shape[1] == N_TOK and x.shape[2] == DIM

    w = host_prep(x, ln1_g, ln1_b, qkv_w, proj_w, proj_b, ln2_g, ln2_b,
                  fc1_w, fc1_b, fc2_w, fc2_b)

    if "nc" not in _CACHE:
        _CACHE["nc"] = build_bass()
    nc = _CACHE["nc"]

    in_maps = [dict(w, x=np.ascontiguousarray(x[i])) for i in range(B)]
    res = run_bass_kernel_spmd(nc, in_maps, core_ids=list(range(B)))
    out = np.stack([res.results[i]["out"] for i in range(B)], axis=0)
    if _want_results:
        return out, res
    return out
